# revision 1
# baseline (speedup 1.0000x reference)
"""Trainium2 Bass kernel for nn_Encoder (3-layer GCN + BatchNorm + MLP head).

Sharding: nodes are sharded across the 8 cores (each core owns a 6250-node
slice of the 50000 per-graph nodes, for all 32 graphs at full 192-float
width).  Per layer:
  - transform + BN-apply + ReLU are computed on the local node slice
    (feature-major [96, nodes] layout, PE for the 6x6 block-diag transform)
  - the message table m (dis-scaled, bf16, rows padded to 512B) is
    AllGather'd into HBM
  - messages are fetched with dma_gather (512B elements) via a per-core
    compacted row table (int16 index limit), in fixed 512-token blocks per
    128-destination chunk
  - the segment-sum (scatter-add) is a TensorEngine matmul against
    host-built fp8 one-hot matrices -> fp32 PSUM, landing feature-major
  - BN statistics are pooled with a 12-float AllReduce
The MLP head contracts over the 300000-dim axis with lw1 row-sharded per
core (bf16) and a [256,32] AllReduce, tail layers replicated.
"""
import os
import numpy as np
import ml_dtypes

N = 50000
B = 32
E = 150000
DIM = 3
H = 6
NC = 8
NS = N // NC            # 6250 nodes per core
NSP = 6272              # padded to 49*128
NBLK = NSP // 128       # 49 dst chunks
EW = 256                # padded bf16 row width (512 bytes)
W = B * H               # 192 payload floats per row
NB = B * N
EPS = 1e-5
CALL = 1024             # gather tokens per call (8 chunks; SWDGE ring limit)

_cache = {}


def _wrap_idx(arr):
    """[n] int array -> [128, n/16] int16 device layout (16-wrap, replicated
    for the 8 Q7 cores)."""
    n = len(arr)
    assert n % 16 == 0
    w = arr.reshape(n // 16, 16).T.astype(np.int16)
    return np.ascontiguousarray(np.tile(w, (8, 1)))


def _build_plan(edge_base):
    """Host-side index preprocessing. Returns uniform shapes + per-core data."""
    row = np.asarray(edge_base[0], dtype=np.int64)
    col = np.asarray(edge_base[1], dtype=np.int64)
    deg = (np.bincount(col, minlength=N) + 1).astype(np.float32)

    cores = []
    for k in range(NC):
        sel = (col // NS) == k
        src = row[sel]
        dstl = col[sel] - NS * k
        order = np.argsort(dstl, kind="stable")
        src, dstl = src[order], dstl[order]
        # block = dst chunk of 128
        blk = dstl // 128
        starts = np.searchsorted(blk, np.arange(NBLK))
        ends = np.searchsorted(blk, np.arange(NBLK) + 1)
        cores.append((src, dstl, starts, ends))

    maxcnt = max(int((e - s).max()) for (_, _, s, e) in
                 [(c[0], c[1], c[2], c[3]) for c in cores])
    tok_blk = max(512, 128 * ((maxcnt + 127) // 128))
    tok = NBLK * tok_blk
    nchunk = tok // 128

    # compact table sizing (uniform across cores)
    mrow_all = []
    for k in range(NC):
        src = cores[k][0]
        mr = (src // NS) * NSP + (src % NS)
        mrow_all.append(np.unique(mr))
    max_lo = max(int((u < 32768).sum()) for u in mrow_all)
    max_hi = max(int((u >= 32768).sum()) for u in mrow_all)
    LP = CALL * ((max_lo + CALL - 1) // CALL)
    HP = CALL * ((max_hi + CALL - 1) // CALL)
    CT = LP + HP

    per_core = []
    for k in range(NC):
        src, dstl, starts, ends = cores[k]
        uniq = mrow_all[k]
        lo = uniq[uniq < 32768]
        hi = uniq[uniq >= 32768]
        clo = np.zeros(LP, np.int64)
        clo[:len(lo)] = lo
        chi = np.zeros(HP, np.int64)
        chi[:len(hi)] = hi - 32768
        pos = np.zeros(NC * NSP, np.int64)
        pos[lo] = np.arange(len(lo))
        pos[hi] = LP + np.arange(len(hi))

        gmsg = np.zeros(tok, np.int64)
        oh = np.zeros((tok, 128), np.float32)
        mr = (src // NS) * NSP + (src % NS)
        for j in range(NBLK):
            s, e = int(starts[j]), int(ends[j])
            n = e - s
            base = j * tok_blk
            gmsg[base:base + n] = pos[mr[s:e]]
            oh[base + np.arange(n), dstl[s:e] - 128 * j] = 1.0
        oh_dev = oh.reshape(nchunk, 128, 128).transpose(1, 0, 2)
        per_core.append({
            "gmsg": _wrap_idx(gmsg),
            "clo": _wrap_idx(clo),
            "chi": _wrap_idx(chi),
            "oh": np.ascontiguousarray(oh_dev.astype(ml_dtypes.float8_e4m3)),
        })

    return {
        "deg": deg, "tok_blk": tok_blk, "tok": tok, "nchunk": nchunk,
        "LP": LP, "HP": HP, "CT": CT, "per_core": per_core,
    }


def _build_nc(plan):
    import concourse.bacc as bacc
    import concourse.mybir as mybir
    import concourse.tile as tile

    dt = mybir.dt
    AF = mybir.ActivationFunctionType
    ALU = mybir.AluOpType
    TOK = plan["tok"]
    TOK_BLK = plan["tok_blk"]
    NCHUNK = plan["nchunk"]
    LP, HP, CT = plan["LP"], plan["HP"], plan["CT"]
    BLK_PER_W = 4                      # dst chunks per psum window
    NW = (NBLK + BLK_PER_W - 1) // BLK_PER_W   # 13 windows
    CPB = TOK_BLK // 128               # msg chunks per block

    nc = bacc.Bacc("TRN2", target_bir_lowering=False, debug=False,
                   num_devices=NC, enable_asserts=False, num_swdge_queues=2)

    # ---------------- I/O ----------------
    def inp(name, shape, d):
        return nc.dram_tensor(name, shape, d, kind="ExternalInput")

    x0 = inp("x0", [48, NSP], dt.float32)
    x1 = inp("x1", [48, NSP], dt.float32)
    degb = inp("degb", [96, NSP], dt.float32)
    gmsg = inp("gmsg", [128, TOK // 16], dt.int16)
    clo = inp("clo", [128, LP // 16], dt.int16)
    chi = inp("chi", [128, HP // 16], dt.int16)
    oh_in = inp("oh", [128, NCHUNK, 128], dt.float8e4)
    bw = [inp("bw1", [48, 96], dt.float32),
          inp("bw2", [96, 96], dt.float32),
          inp("bw3", [96, 96], dt.float32)]
    i96 = inp("i96", [96, 96], dt.float32)
    sel = inp("sel", [96, 6], dt.float32)
    selT = inp("selT", [6, 96], dt.float32)
    gam = [inp(f"g{i}", [6, 1], dt.float32) for i in (1, 2, 3)]
    bet = [inp(f"be{i}", [6, 1], dt.float32) for i in (1, 2, 3)]
    lw1p = inp("lw1p", [H, NSP, 256], dt.bfloat16)
    lw2r = inp("lw2r", [128, 2, 128], dt.float32)
    lw3 = inp("lw3", [128, 64], dt.float32)
    lw4 = inp("lw4", [64, 32], dt.float32)
    lb1h = inp("lb1h", [128, 2], dt.float32)
    lb2c = inp("lb2c", [128, 1], dt.float32)
    lb3c = inp("lb3c", [64, 1], dt.float32)
    lb4c = inp("lb4c", [32, 1], dt.float32)
    out_d = nc.dram_tensor("out", [B, 32], dt.float32, kind="ExternalOutput")
    dbg_d = nc.dram_tensor("dbg", [128, 512], dt.float32, kind="ExternalOutput")
    STAGE = int(os.environ.get("KSTAGE", "7"))
    KSUB = os.environ.get("KSUB", "z")

    m_hbm = nc.dram_tensor("m_hbm", [NSP, EW], dt.bfloat16, kind="Internal")
    m_full = nc.dram_tensor("m_full", [NC * NSP, EW], dt.bfloat16,
                            kind="Internal", addr_space="Shared")
    cmp_hbm = nc.dram_tensor("cmp_hbm", [CT, EW], dt.bfloat16, kind="Internal")
    st_in = [nc.dram_tensor(f"st_in{i}", [6, 2], dt.float32, kind="Internal")
             for i in range(3)]
    st_out = [nc.dram_tensor(f"st_out{i}", [6, 2], dt.float32, kind="Internal",
                             addr_space="Shared") for i in range(3)]
    mlp_in = nc.dram_tensor("mlp_in", [2, 128, 32], dt.float32, kind="Internal")
    mlp_out = nc.dram_tensor("mlp_out", [2, 128, 32], dt.float32,
                             kind="Internal", addr_space="Shared")

    groups = [list(range(NC))]

    with tile.TileContext(nc) as tc:
        with (
            tc.tile_pool(name="const", bufs=1) as cpool,
            tc.tile_pool(name="ho", bufs=1) as ho_pool,
            tc.tile_pool(name="mfm", bufs=1) as mfm_pool,
            tc.tile_pool(name="mnm", bufs=1) as mnm_pool,
            tc.tile_pool(name="msg", bufs=4) as msg_pool,
            tc.tile_pool(name="ohp", bufs=3) as oh_pool,
            tc.tile_pool(name="ysc", bufs=2) as y_pool,
            tc.tile_pool(name="acc", bufs=4) as acc_pool,
            tc.tile_pool(name="st", bufs=1) as st_pool,
            tc.tile_pool(name="t6", bufs=1) as t6_pool,
            tc.tile_pool(name="stg", bufs=2) as stg_pool,
            tc.tile_pool(name="psA", bufs=4, space="PSUM") as psA,
            tc.tile_pool(name="ps1", bufs=2, space="PSUM") as ps1,
            tc.tile_pool(name="psD", bufs=1, space="PSUM") as psD,
        ):
            # ---------- setup ----------
            gmsg_sb = cpool.tile([128, TOK // 16], dt.int16)
            nc.sync.dma_start(gmsg_sb[:], gmsg[:])
            clo_sb = cpool.tile([128, LP // 16], dt.int16)
            nc.sync.dma_start(clo_sb[:], clo[:])
            chi_sb = cpool.tile([128, HP // 16], dt.int16)
            nc.sync.dma_start(chi_sb[:], chi[:])
            bw_sb = []
            for i in range(3):
                t = cpool.tile([48 if i == 0 else 96, 96], dt.float32,
                               tag=f"bw{i}", name=f"bw_sb{i}")
                nc.sync.dma_start(t[:], bw[i][:])
                bw_sb.append(t)
            i96_sb = cpool.tile([96, 96], dt.float32)
            nc.sync.dma_start(i96_sb[:], i96[:])
            sel_sb = cpool.tile([96, 6], dt.float32)
            nc.sync.dma_start(sel_sb[:], sel[:])
            selT_sb = cpool.tile([6, 96], dt.float32)
            nc.sync.dma_start(selT_sb[:], selT[:])
            gam_sb, bet_sb = [], []
            for i in range(3):
                g_t = cpool.tile([6, 1], dt.float32, tag=f"gam{i}", name=f"gam_sb{i}")
                nc.sync.dma_start(g_t[:], gam[i][:])
                gam_sb.append(g_t)
                b_t = cpool.tile([6, 1], dt.float32, tag=f"bet{i}", name=f"bet_sb{i}")
                nc.sync.dma_start(b_t[:], bet[i][:])
                bet_sb.append(b_t)

            eps_sb = cpool.tile([6, 1], dt.float32, name="eps_sb")
            nc.vector.memset(eps_sb[:], EPS)
            # dis = 1/sqrt(deg), materialized [96, NSP]
            dis_sb = cpool.tile([96, NSP], dt.float32)
            nc.sync.dma_start(dis_sb[:], degb[:])
            nc.scalar.activation(dis_sb[:], dis_sb[:], AF.Sqrt)
            nc.vector.reciprocal(dis_sb[:], dis_sb[:])

            # x feature-major
            h_t = [ho_pool.tile([48, NSP], dt.float32, tag=f"ho{u}", name=f"x_sb{u}")
                   for u in range(2)]
            nc.sync.dma_start(h_t[0][:], x0[:])
            nc.sync.dma_start(h_t[1][:], x1[:])

            o_t = [None, None]

            # ================= conv layers =================
            n_layers = 3 if STAGE >= 5 else 1
            for L in range(n_layers):
                kin = 48 if L == 0 else 96
                # --- A: dis-scale h (in place) ---
                if KSUB < "b":
                    nc.sync.dma_start(dbg_d.ap()[0:96, :], dis_sb[:, 0:512])
                    break
                for u in range(2):
                    nc.vector.tensor_mul(h_t[u][:], h_t[u][:],
                                         dis_sb[0:kin, :])
                if KSUB < "c":
                    nc.sync.dma_start(dbg_d.ap()[0:96, :], dis_sb[:, 0:512])
                    break
                # --- B: m_fm = BW^T @ h (bf16) ---
                m_fm = [mfm_pool.tile([96, NSP], dt.bfloat16, tag=f"mfm{u}", name=f"mfm_L{L}_{u}")
                        for u in range(2)]
                for u in range(2):
                    for c0 in range(0, NSP, 512):
                        cw = min(512, NSP - c0)
                        pt = ps1.tile([96, 512], dt.float32, tag="ps1", name="ptb")
                        nc.tensor.matmul(pt[:, :cw], bw_sb[L][:],
                                         h_t[u][:, c0:c0 + cw],
                                         start=True, stop=True)
                        nc.vector.tensor_copy(m_fm[u][:, c0:c0 + cw],
                                              pt[:, :cw])
                if KSUB < "d":
                    nc.sync.dma_start(dbg_d.ap()[0:96, :], dis_sb[:, 0:512])
                    break
                # --- C: m_nm (node-major bf16) ---
                m_nm = mnm_pool.tile([128, NBLK, EW], dt.bfloat16, tag="mnm", name=f"mnm_L{L}")
                if L == 0:
                    nc.vector.memset(m_nm[:, :, W:EW], 0.0)
                for u in range(2):
                    for b0 in range(0, NBLK, 5):
                        nb = min(5, NBLK - b0)
                        pt = ps1.tile([128, 480], dt.float32, tag="ps1", name="ptc")
                        for i in range(nb):
                            c = b0 + i
                            nc.tensor.matmul(
                                pt[:, 96 * i:96 * (i + 1)],
                                h_t[u][:, 128 * c:128 * (c + 1)],
                                bw_sb[L][:], start=True, stop=True)
                        src = pt[:, :96 * nb].rearrange("p (c f) -> p c f", f=96)
                        nc.vector.tensor_copy(
                            m_nm[:, b0:b0 + nb, 96 * u:96 * (u + 1)], src)
                if KSUB < "e":
                    nc.sync.dma_start(dbg_d.ap()[0:96, :], dis_sb[:, 0:512])
                    break
                # --- D: write + AllGather ---
                nc.sync.dma_start(
                    m_hbm.ap().rearrange("(c p) e -> p c e", p=128), m_nm[:])
                if STAGE < 2:
                    dv = m_nm[:, 0:2, :].rearrange("p c e -> p (c e)")
                    nc.sync.dma_start(dbg_d.ap()[:, 0:256], dv[:, 0:512].bitcast(dt.float32))
                    break
                nc.gpsimd.collective_compute(
                    "AllGather", ALU.bypass, replica_groups=groups,
                    ins=[m_hbm.ap()], outs=[m_full.ap()])
                if KSUB < "g":
                    nc.sync.dma_start(dbg_d.ap()[0:96, :], dis_sb[:, 0:512])
                    break
                SKIP_WB = KSUB == "g"
                # --- E: compact table ---
                for ci in range(LP // CALL):
                    t = msg_pool.tile([128, CALL // 128, EW], dt.bfloat16,
                                      tag="msg", name=f"cmp_L{L}_{ci}")
                    nc.gpsimd.dma_gather(
                        t[:], m_full.ap()[0:32768, :],
                        clo_sb[:, ci * (CALL // 16):(ci + 1) * (CALL // 16)],
                        num_idxs=CALL, num_idxs_reg=CALL, elem_size=EW,
                        queue_num=ci % 2)
                    if not SKIP_WB:
                        nc.sync.dma_start(
                            cmp_hbm.ap()[ci * CALL:(ci + 1) * CALL, :]
                            .rearrange("(s p) e -> p s e", p=128), t[:])
                for ci in range(HP // CALL):
                    t = msg_pool.tile([128, CALL // 128, EW], dt.bfloat16,
                                      tag="msg", name=f"cmp_L{L}_{ci}")
                    nc.gpsimd.dma_gather(
                        t[:], m_full.ap()[32768:NC * NSP, :],
                        chi_sb[:, ci * (CALL // 16):(ci + 1) * (CALL // 16)],
                        num_idxs=CALL, num_idxs_reg=CALL, elem_size=EW,
                        queue_num=ci % 2)
                    if not SKIP_WB:
                        nc.sync.dma_start(
                            cmp_hbm.ap()[LP + ci * CALL:LP + (ci + 1) * CALL, :]
                            .rearrange("(s p) e -> p s e", p=128), t[:])
                if STAGE < 3:
                    tdbg = msg_pool.tile([128, 2, EW], dt.bfloat16, tag="msg",
                                         name="tdbg")
                    nc.sync.dma_start(
                        tdbg[:], cmp_hbm.ap()[0:256, :]
                        .rearrange("(s p) e -> p s e", p=128))
                    nc.sync.dma_start(dbg_d.ap()[:, 0:256],
                                      tdbg[:].rearrange("p s e -> p (s e)")
                                      [:, 0:512].bitcast(dt.float32))
                    break
                # --- F: message gather + matmul segsum ---
                o_t[0] = ho_pool.tile([96, NSP], dt.float32, tag="ho0", name=f"o_L{L}_0")
                o_t[1] = ho_pool.tile([96, NSP], dt.float32, tag="ho1", name=f"o_L{L}_1")
                S_t = st_pool.tile([96, 4 * NW], dt.float32, tag="S")
                msg_tiles = {}
                oh_tiles = {}
                CPC = CALL // 128       # chunks per call
                ncalls = (NCHUNK + CPC - 1) // CPC
                for ci in range(ncalls):
                    nch = min(CPC, NCHUNK - ci * CPC)
                    t = msg_pool.tile([128, nch, EW], dt.bfloat16, tag="msg", name=f"msg_L{L}_{ci}")
                    nc.gpsimd.dma_gather(
                        t[:], cmp_hbm.ap(),
                        gmsg_sb[:, ci * (CALL // 16):ci * (CALL // 16) + nch * 8],
                        num_idxs=nch * 128, num_idxs_reg=nch * 128,
                        elem_size=EW, queue_num=ci % 2)
                    msg_tiles[ci] = t
                noh = (NCHUNK + 15) // 16
                for ti in range(noh):
                    nch = min(16, NCHUNK - ti * 16)
                    t = oh_pool.tile([128, nch, 128], dt.float8e4, tag="oh", name=f"oh_L{L}_{ti}")
                    nc.sync.dma_start(t[:], oh_in[:, ti * 16:ti * 16 + nch, :])
                    oh_tiles[ti] = t
                FSUB = os.environ.get("FSUB", "z")
                if FSUB < "j":
                    nc.sync.dma_start(dbg_d.ap()[0:96, :], dis_sb[:, 0:512])
                    break
                for w in range(NW):
                    jlo = w * BLK_PER_W
                    jhi = min(jlo + BLK_PER_W, NBLK)
                    pw = [psA.tile([96, 512], dt.float32, tag="psA", name=f"pw_L{L}_{w}_{uu}")
                          for uu in range(2)]
                    for j in range(jlo, jhi):
                        for c4 in range(CPB):
                            q = CPB * j + c4
                            mt = msg_tiles[q // CPC]
                            ot = oh_tiles[q // 16]
                            for u in range(2):
                                nc.tensor.matmul(
                                    pw[u][:, 128 * (j - jlo):128 * (j - jlo + 1)],
                                    mt[:, q % CPC, 96 * u:96 * (u + 1)],
                                    ot[:, q % 16, :],
                                    start=(c4 == 0), stop=(c4 == CPB - 1))
                    c0 = 512 * w
                    cw = min(512, NS - c0)   # stats only over real 6250 cols
                    cwf = min(512, NSP - c0)
                    for u in range(2):
                        y = y_pool.tile([96, 512], dt.float32, tag="y")
                        nc.vector.tensor_add(y[:, :cwf], pw[u][:, :cwf],
                                             m_fm[u][:, c0:c0 + cwf])
                        nc.vector.tensor_mul(o_t[u][:, c0:c0 + cwf],
                                             y[:, :cwf],
                                             dis_sb[:, c0:c0 + cwf])
                        nc.vector.tensor_reduce(
                            S_t[:, 2 * w + u:2 * w + u + 1],
                            o_t[u][:, c0:c0 + cw],
                            axis=mybir.AxisListType.X, op=ALU.add)
                        y2 = y_pool.tile([96, 512], dt.float32, tag="y")
                        acc = acc_pool.tile([96, 1], dt.float32, tag="acc",
                                            name=f"acc_{L}_{w}_{u}")
                        nc.scalar.activation(
                            y2[:, :cw], o_t[u][:, c0:c0 + cw], AF.Square,
                            accum_out=acc[:])
                        nc.vector.tensor_copy(
                            S_t[:, 2 * (NW + w) + u:2 * (NW + w) + u + 1],
                            acc[:])
                if STAGE < 4:
                    nc.sync.dma_start(dbg_d.ap()[0:96, :], o_t[0][:, 0:512])
                    break
                # --- G: BN stats ---
                st2 = st_pool.tile([96, 4], dt.float32, tag="st2")
                for u in range(2):
                    nc.vector.tensor_reduce(
                        st2[:, u:u + 1],
                        S_t[:, :2 * NW].rearrange("p (w u) -> p u w", u=2)[:, u, :],
                        axis=mybir.AxisListType.X, op=ALU.add)
                    nc.vector.tensor_reduce(
                        st2[:, 2 + u:3 + u],
                        S_t[:, 2 * NW:4 * NW].rearrange("p (w u) -> p u w", u=2)[:, u, :],
                        axis=mybir.AxisListType.X, op=ALU.add)
                pst = psD.tile([6, 2], dt.float32, tag="pst")
                for u in range(2):
                    nc.tensor.matmul(
                        pst[:],
                        sel_sb[:],
                        st2[:, :].rearrange("p (a u) -> p u a", u=2)[:, u, :],
                        start=(u == 0), stop=(u == 1))
                stt = t6_pool.tile([6, 2], dt.float32, tag="stt")
                nc.vector.tensor_copy(stt[:], pst[:])
                nc.sync.dma_start(st_in[L].ap(), stt[:])
                nc.gpsimd.collective_compute(
                    "AllReduce", ALU.add, replica_groups=groups,
                    ins=[st_in[L].ap()], outs=[st_out[L].ap()])
                sto = t6_pool.tile([6, 2], dt.float32, tag="sto")
                nc.sync.dma_start(sto[:], st_out[L].ap())
                mu = t6_pool.tile([6, 1], dt.float32, tag="mu")
                nc.vector.tensor_scalar_mul(mu[:], sto[:, 0:1], 1.0 / NB)
                var = t6_pool.tile([6, 1], dt.float32, tag="var")
                nc.vector.tensor_scalar_mul(var[:], sto[:, 1:2], 1.0 / NB)
                musq = t6_pool.tile([6, 1], dt.float32, tag="musq")
                nc.vector.tensor_mul(musq[:], mu[:], mu[:])
                nc.vector.tensor_sub(var[:], var[:], musq[:])
                nc.scalar.activation(var[:], var[:], AF.Sqrt, bias=eps_sb[:].opt())
                nc.vector.reciprocal(var[:], var[:])     # var := 1/sigma
                ab6 = t6_pool.tile([6, 2], dt.float32, tag="ab6")
                nc.vector.tensor_mul(ab6[:, 0:1], gam_sb[L][:], var[:])
                nc.vector.tensor_mul(musq[:], mu[:], ab6[:, 0:1])
                nc.vector.tensor_sub(ab6[:, 1:2], bet_sb[L][:], musq[:])
                pab = psD.tile([96, 2], dt.float32, tag="pab")
                nc.tensor.matmul(pab[:], selT_sb[:], ab6[:],
                                 start=True, stop=True)
                ab = st_pool.tile([96, 2], dt.float32, tag="ab")
                nc.vector.tensor_copy(ab[:], pab[:])
                # --- H: BN apply + relu (in place on o, which becomes h) ---
                for u in range(2):
                    nc.scalar.activation(o_t[u][:], o_t[u][:], AF.Relu,
                                         bias=ab[:, 1:2].opt(),
                                         scale=ab[:, 0:1].opt())
                h_t = [o_t[0], o_t[1]]

            # ================= MLP head =================
            do_mlp = STAGE >= 6
            if do_mlp:
                # o -> node-major bf16 (transpose via PE with identity)
                o_bf = mnm_pool.tile([128, NBLK, EW], dt.bfloat16, tag="mnm")
                for u in range(2):
                    for b0 in range(0, NBLK, 5):
                        nb = min(5, NBLK - b0)
                        pt = ps1.tile([128, 480], dt.float32, tag="ps1", name="pto")
                        for i in range(nb):
                            c = b0 + i
                            nc.tensor.matmul(pt[:, 96 * i:96 * (i + 1)],
                                             h_t[u][:, 128 * c:128 * (c + 1)],
                                             i96_sb[:], start=True, stop=True)
                        src = pt[:, :96 * nb].rearrange("p (c f) -> p c f", f=96)
                        nc.vector.tensor_copy(
                            o_bf[:, b0:b0 + nb, 96 * u:96 * (u + 1)], src)

                zt = [psD.tile([128, 32], dt.float32, tag="pst", name="zt0"),
                      psD.tile([128, 32], dt.float32, tag="pab", name="zt1")]
                nstg = (H * NBLK + 15) // 16
                stg_tiles = {}
                lw1v = lw1p.ap().rearrange("f (c p) e -> (f c) p e", p=128)
                for si in range(nstg):
                    nch = min(16, H * NBLK - si * 16)
                    t = stg_pool.tile([128, nch, 256], dt.bfloat16, tag="stg", name=f"stg{si}")
                    nc.sync.dma_start(
                        t[:], lw1v[si * 16:si * 16 + nch]
                        .rearrange("c p e -> p c e"))
                    stg_tiles[si] = t
                for f in range(H):
                    for c in range(NBLK):
                        fc = f * NBLK + c
                        st_t = stg_tiles[fc // 16]
                        rhs = o_bf[:, c, 0:W].rearrange("p (g f) -> p f g", f=H)[:, f, :]
                        for h2 in range(2):
                            nc.tensor.matmul(
                                zt[h2][:],
                                st_t[:, fc % 16, 128 * h2:128 * (h2 + 1)],
                                rhs, start=(fc == 0), stop=(fc == H * NBLK - 1))
                # evict z psums to SBUF then to dram for AllReduce
                zc = [st_pool.tile([128, 32], dt.float32, tag=f"zc{h2}", name=f"zc{h2}")
                      for h2 in range(2)]
                for h2 in range(2):
                    nc.vector.tensor_copy(zc[h2][:], zt[h2][:])
                    nc.sync.dma_start(mlp_in.ap()[h2], zc[h2][:])
                nc.gpsimd.collective_compute(
                    "AllReduce", ALU.add, replica_groups=groups,
                    ins=[mlp_in.ap()], outs=[mlp_out.ap()])
                lb1_sb = cpool.tile([128, 2], dt.float32)
                nc.sync.dma_start(lb1_sb[:], lb1h[:])
                lw2_sb = cpool.tile([128, 2, 128], dt.float32)
                nc.sync.dma_start(lw2_sb[:], lw2r[:])
                lw3_sb = cpool.tile([128, 64], dt.float32)
                nc.sync.dma_start(lw3_sb[:], lw3[:])
                lw4_sb = cpool.tile([64, 32], dt.float32)
                nc.sync.dma_start(lw4_sb[:], lw4[:])
                lb2_sb = cpool.tile([128, 1], dt.float32)
                nc.sync.dma_start(lb2_sb[:], lb2c[:])
                lb3_sb = cpool.tile([64, 1], dt.float32)
                nc.sync.dma_start(lb3_sb[:], lb3c[:])
                lb4_sb = cpool.tile([32, 1], dt.float32)
                nc.sync.dma_start(lb4_sb[:], lb4c[:])

                h1 = [st_pool.tile([128, 32], dt.float32, tag=f"h1{h2}", name=f"h1_{h2}")
                      for h2 in range(2)]
                for h2 in range(2):
                    nc.sync.dma_start(h1[h2][:], mlp_out.ap()[h2])
                    nc.scalar.activation(h1[h2][:], h1[h2][:], AF.Relu,
                                         bias=lb1_sb[:, h2:h2 + 1].opt())
                p2 = psD.tile([128, 32], dt.float32, tag="pst")
                for h2 in range(2):
                    nc.tensor.matmul(p2[:], lw2_sb[:, h2, :], h1[h2][:],
                                     start=(h2 == 0), stop=(h2 == 1))
                h2x = st_pool.tile([128, 32], dt.float32, tag="h2x")
                nc.scalar.activation(h2x[:], p2[:], AF.Relu, bias=lb2_sb[:].opt())
                p3 = psD.tile([64, 32], dt.float32, tag="pab")
                nc.tensor.matmul(p3[:], lw3_sb[:], h2x[:], start=True, stop=True)
                h3 = st_pool.tile([64, 32], dt.float32, tag="h3")
                nc.scalar.activation(h3[:], p3[:], AF.Relu, bias=lb3_sb[:].opt())
                p4 = psD.tile([32, 32], dt.float32, tag="pst")
                nc.tensor.matmul(p4[:], lw4_sb[:], h3[:], start=True, stop=True)
                o4 = st_pool.tile([32, 32], dt.float32, tag="o4")
                nc.vector.tensor_scalar_add(o4[:], p4[:], lb4_sb[:].opt())
                nc.sync.dma_start(out_d.ap().rearrange("g c -> c g"), o4[:])

    nc.compile()
    return nc


def _make_in_maps(plan, inputs):
    f32 = np.float32
    x = np.asarray(inputs["x"], f32)
    W1 = np.asarray(inputs["W1"], f32)
    W2 = np.asarray(inputs["W2"], f32)
    W3 = np.asarray(inputs["W3"], f32)
    lw1 = np.asarray(inputs["lw1"], f32)
    lw2 = np.asarray(inputs["lw2"], f32)
    lw3 = np.asarray(inputs["lw3"], f32)
    lw4 = np.asarray(inputs["lw4"], f32)

    bw1 = np.kron(np.eye(16, dtype=f32), W1)            # [48, 96]
    bw2 = np.kron(np.eye(16, dtype=f32), W2)            # [96, 96]
    bw3 = np.kron(np.eye(16, dtype=f32), W3)
    i96 = np.eye(96, dtype=f32)
    sel = np.tile(np.eye(6, dtype=f32), (16, 1))        # [96, 6]
    selT = np.ascontiguousarray(sel.T)                  # [6, 96]
    lw2r = np.ascontiguousarray(
        lw2.reshape(2, 128, 128).transpose(1, 0, 2))    # [128, 2, 128]
    lb1h = np.ascontiguousarray(
        np.asarray(inputs["lb1"], f32).reshape(2, 128).T)  # [128, 2]

    deg = plan["deg"]
    xg = x.reshape(B, N, DIM)

    common = {
        "bw1": bw1, "bw2": bw2, "bw3": bw3, "i96": i96,
        "sel": sel, "selT": selT,
        "g1": np.asarray(inputs["g1"], f32).reshape(6, 1),
        "be1": np.asarray(inputs["be1"], f32).reshape(6, 1),
        "g2": np.asarray(inputs["g2"], f32).reshape(6, 1),
        "be2": np.asarray(inputs["be2"], f32).reshape(6, 1),
        "g3": np.asarray(inputs["g3"], f32).reshape(6, 1),
        "be3": np.asarray(inputs["be3"], f32).reshape(6, 1),
        "lw2r": lw2r, "lw3": lw3, "lw4": lw4,
        "lb1h": lb1h,
        "lb2c": np.asarray(inputs["lb2"], f32).reshape(128, 1),
        "lb3c": np.asarray(inputs["lb3"], f32).reshape(64, 1),
        "lb4c": np.asarray(inputs["lb4"], f32).reshape(32, 1),
    }

    in_maps = []
    for k in range(NC):
        pc = plan["per_core"][k]
        # x feature-major: xq[u][g*3+fi, n]
        xs = xg[:, NS * k:NS * (k + 1), :]              # [32, 6250, 3]
        xq = np.zeros((2, 48, NSP), f32)
        for u in range(2):
            blkv = xs[16 * u:16 * (u + 1)].transpose(0, 2, 1)  # [16, 3, 6250]
            xq[u, :, :NS] = blkv.reshape(48, NS)
        degb = np.ones((96, NSP), f32)
        degb[:, :NS] = deg[NS * k:NS * (k + 1)][None, :]
        # lw1 rows f-major per core, zero-padded nodes
        lw1s = lw1[NS * k * H:NS * (k + 1) * H].reshape(NS, H, 256)
        lw1p = np.zeros((H, NSP, 256), ml_dtypes.bfloat16)
        lw1p[:, :NS, :] = lw1s.transpose(1, 0, 2).astype(ml_dtypes.bfloat16)
        m = dict(common)
        m.update({
            "x0": np.ascontiguousarray(xq[0]),
            "x1": np.ascontiguousarray(xq[1]),
            "degb": degb,
            "gmsg": pc["gmsg"], "clo": pc["clo"], "chi": pc["chi"],
            "oh": pc["oh"], "lw1p": lw1p,
        })
        in_maps.append(m)
    return in_maps


def _get(edge_base):
    key = hash(np.asarray(edge_base).tobytes())
    if key not in _cache:
        plan = _build_plan(np.asarray(edge_base))
        nc = _build_nc(plan)
        _cache[key] = (plan, nc)
    return _cache[key]


def kernel(**inputs):
    from concourse.bass_utils import run_bass_kernel_spmd
    assert int(inputs["num_graphs"]) == B and int(inputs["num_nodes"]) == N
    plan, nc = _get(inputs["edge_base"])
    in_maps = _make_in_maps(plan, inputs)
    trace = os.environ.get("KERNEL_TRACE", "0") == "1"
    res = run_bass_kernel_spmd(nc, in_maps, core_ids=list(range(NC)),
                               trace=trace)
    kernel.last_result = res
    return np.ascontiguousarray(res.results[0]["out"])



# revision 3
# speedup vs baseline: 1.2371x; 1.2371x over previous
"""Trainium2 Bass kernel for nn_Encoder (3-layer GCN + BatchNorm + MLP head).

Sharding: nodes sharded across 8 cores (6250-node slices, all 32 graphs at
full 192-float width).  Per layer:
  - transform runs in bf16 on the PE; the node-major message table is split
    into two half-tables (3200 + 3072 rows/core) that are AllGather'd
    separately so the second AllGather and the first gather/segment-sum pass
    overlap
  - messages are fetched straight from the gathered half-tables with
    dma_gather (512B rows, int16 indices fit because each half-table has
    <32768 rows) over 4 SWDGE queues; no intermediate compaction
  - the segment-sum is a PE matmul against host-built fp8 one-hot matrices;
    pass A (table-0 tokens) evicts partial sums (+self term) to SBUF, pass B
    (table-1 tokens) finishes, scales by dis, and computes BN stats
  - BN statistics are pooled with a 12-float AllReduce
The MLP head contracts the 300000-dim axis with lw1 row-sharded per core
(bf16, streamed through SBUF with prefetch) and a [256,32] AllReduce.
"""
import os
import numpy as np
import ml_dtypes

N = 50000
B = 32
E = 150000
DIM = 3
H = 6
NC = 8
NS = N // NC            # 6250 nodes per core
NSP = 6272              # padded to 49*128
NBLK = NSP // 128       # 49 dst chunks
H0C = 25                # chunks in half-table 0
H1C = NBLK - H0C        # 24
H0 = H0C * 128          # 3200 rows/core in table 0
H1 = H1C * 128          # 3072 rows/core in table 1
EW = 256                # padded bf16 row width (512 bytes)
W = B * H               # 192 payload floats per row
NB = B * N
EPS = 1e-5
CALL = 1024             # gather tokens per call (SWDGE ring limit)
CPC = CALL // 128       # msg chunks per call
NQ = 4                  # SWDGE queues
BLK_PER_W = 4           # dst chunks per psum window
NW = (NBLK + BLK_PER_W - 1) // BLK_PER_W   # 13 windows

_cache = {}


def _wrap_idx(arr):
    """[n] int array -> [128, n/16] int16 device layout (16-wrap, replicated
    for the 8 Q7 cores)."""
    n = len(arr)
    assert n % 16 == 0
    w = arr.reshape(n // 16, 16).T.astype(np.int16)
    return np.ascontiguousarray(np.tile(w, (8, 1)))


def _build_plan(edge_base):
    """Host-side index preprocessing. Returns uniform shapes + per-core data."""
    row = np.asarray(edge_base[0], dtype=np.int64)
    col = np.asarray(edge_base[1], dtype=np.int64)
    deg = (np.bincount(col, minlength=N) + 1).astype(np.float32)
    dis = 1.0 / np.sqrt(deg)

    # per-core edge lists, split by source half-table
    cores = []
    cnt = np.zeros((2, NC, NBLK), np.int64)
    for k in range(NC):
        sel = (col // NS) == k
        src = row[sel]
        dstl = col[sel] - NS * k
        j = src // NS
        off = src - j * NS
        t = (off >= H0).astype(np.int64)
        trow = np.where(t == 0, j * H0 + off, j * H1 + (off - H0))
        b = dstl // 128
        cores.append((t, b, trow, dstl))
        for tt in (0, 1):
            cnt[tt, k] = np.bincount(b[t == tt], minlength=NBLK)

    # uniform (cross-core) chunk counts per (block, table)
    CH = [np.maximum(1, -(-cnt[tt].max(axis=0) // 128)).astype(int)
          for tt in (0, 1)]
    choff = [np.concatenate([[0], np.cumsum(CH[tt])]) for tt in (0, 1)]
    nchunk = [int(CH[tt].sum()) for tt in (0, 1)]
    tok = [nchunk[tt] * 128 for tt in (0, 1)]

    per_core = []
    for k in range(NC):
        t, b, trow, dstl = cores[k]
        gmsg = [np.zeros(tok[tt], np.int64) for tt in (0, 1)]
        oh = [np.zeros((tok[tt], 128), np.float32) for tt in (0, 1)]
        for tt in (0, 1):
            m = t == tt
            bb, rr, dd = b[m], trow[m], dstl[m]
            order = np.argsort(bb, kind="stable")
            bb, rr, dd = bb[order], rr[order], dd[order]
            starts = np.searchsorted(bb, np.arange(NBLK))
            ends = np.searchsorted(bb, np.arange(NBLK) + 1)
            for jb in range(NBLK):
                s, e = int(starts[jb]), int(ends[jb])
                n = e - s
                base = int(choff[tt][jb]) * 128
                assert n <= CH[tt][jb] * 128
                gmsg[tt][base:base + n] = rr[s:e]
                oh[tt][base + np.arange(n), dd[s:e] - 128 * jb] = 1.0
        pc = {}
        for tt in (0, 1):
            oh_dev = (oh[tt].reshape(nchunk[tt], 128, 128)
                      .transpose(1, 0, 2))
            pc[f"gmsg{tt}"] = _wrap_idx(gmsg[tt])
            pc[f"oh{tt}"] = np.ascontiguousarray(
                oh_dev.astype(ml_dtypes.float8_e4m3))
        per_core.append(pc)

    return {
        "dis": dis, "CH": CH, "choff": choff, "nchunk": nchunk, "tok": tok,
        "per_core": per_core,
    }


def _build_nc(plan):
    import concourse.bacc as bacc
    import concourse.mybir as mybir
    import concourse.tile as tile

    dt = mybir.dt
    AF = mybir.ActivationFunctionType
    ALU = mybir.AluOpType
    CH = plan["CH"]
    CHOFF = plan["choff"]
    NCHUNK = plan["nchunk"]
    TOK = plan["tok"]
    NCALL = [-(-TOK[tt] // CALL) for tt in (0, 1)]
    NOHT = [-(-NCHUNK[tt] // 16) for tt in (0, 1)]

    nc = bacc.Bacc("TRN2", target_bir_lowering=False, debug=False,
                   num_devices=NC, enable_asserts=False, num_swdge_queues=NQ)

    # ---------------- I/O ----------------
    def inp(name, shape, d):
        return nc.dram_tensor(name, shape, d, kind="ExternalInput")

    x0 = inp("x0", [48, NSP], dt.bfloat16)
    x1 = inp("x1", [48, NSP], dt.bfloat16)
    disb = inp("disb", [96, NSP], dt.bfloat16)
    gmsg = [inp(f"gmsg{tt}", [128, TOK[tt] // 16], dt.int16) for tt in (0, 1)]
    oh_in = [inp(f"oh{tt}", [128, NCHUNK[tt], 128], dt.float8e4)
             for tt in (0, 1)]
    bw = [inp("bw1", [48, 96], dt.bfloat16),
          inp("bw2", [96, 96], dt.bfloat16),
          inp("bw3", [96, 96], dt.bfloat16)]
    i96 = inp("i96", [96, 96], dt.bfloat16)
    sel = inp("sel", [96, 6], dt.float32)
    selT = inp("selT", [6, 96], dt.float32)
    gam = [inp(f"g{i}", [6, 1], dt.float32) for i in (1, 2, 3)]
    bet = [inp(f"be{i}", [6, 1], dt.float32) for i in (1, 2, 3)]
    lw1p = inp("lw1p", [H, NSP, 256], dt.bfloat16)
    lw2r = inp("lw2r", [128, 2, 128], dt.float32)
    lw3 = inp("lw3", [128, 64], dt.float32)
    lw4 = inp("lw4", [64, 32], dt.float32)
    lb1h = inp("lb1h", [128, 2], dt.float32)
    lb2c = inp("lb2c", [128, 1], dt.float32)
    lb3c = inp("lb3c", [64, 1], dt.float32)
    lb4c = inp("lb4c", [32, 1], dt.float32)
    out_d = nc.dram_tensor("out", [B, 32], dt.float32, kind="ExternalOutput")

    m_hbm = [nc.dram_tensor("m_hbm0", [H0, EW], dt.bfloat16, kind="Internal"),
             nc.dram_tensor("m_hbm1", [H1, EW], dt.bfloat16, kind="Internal")]
    m_full = [nc.dram_tensor("m_full0", [NC * H0, EW], dt.bfloat16,
                             kind="Internal", addr_space="Shared"),
              nc.dram_tensor("m_full1", [NC * H1, EW], dt.bfloat16,
                             kind="Internal", addr_space="Shared")]
    st_in = [nc.dram_tensor(f"st_in{i}", [6, 2], dt.float32, kind="Internal")
             for i in range(3)]
    st_out = [nc.dram_tensor(f"st_out{i}", [6, 2], dt.float32, kind="Internal",
                             addr_space="Shared") for i in range(3)]
    mlp_in = nc.dram_tensor("mlp_in", [2, 128, 32], dt.float32, kind="Internal")
    mlp_out = nc.dram_tensor("mlp_out", [2, 128, 32], dt.float32,
                             kind="Internal", addr_space="Shared")

    groups = [list(range(NC))]
    NSTG = (H * NBLK + 15) // 16       # lw1 staging tiles
    STG_BUFS = 6

    with tile.TileContext(nc) as tc:
        with (
            tc.tile_pool(name="const", bufs=1) as cpool,
            tc.tile_pool(name="ho", bufs=1) as ho_pool,
            tc.tile_pool(name="mfm", bufs=1) as mfm_pool,
            tc.tile_pool(name="mnm", bufs=1) as mnm_pool,
            tc.tile_pool(name="oacc", bufs=1) as oacc_pool,
            tc.tile_pool(name="msg", bufs=6) as msg_pool,
            tc.tile_pool(name="ohp", bufs=4) as oh_pool,
            tc.tile_pool(name="ysc", bufs=3) as y_pool,
            tc.tile_pool(name="acc", bufs=4) as acc_pool,
            tc.tile_pool(name="st", bufs=1) as st_pool,
            tc.tile_pool(name="t6", bufs=1) as t6_pool,
            tc.tile_pool(name="stg", bufs=STG_BUFS) as stg_pool,
            tc.tile_pool(name="psA", bufs=4, space="PSUM") as psA,
            tc.tile_pool(name="ps1", bufs=2, space="PSUM") as ps1,
            tc.tile_pool(name="psD", bufs=1, space="PSUM") as psD,
        ):
            # ---------- setup ----------
            gmsg_sb = []
            for tt in (0, 1):
                t = cpool.tile([128, TOK[tt] // 16], dt.int16,
                               tag=f"gmsg{tt}", name=f"gmsg_sb{tt}")
                nc.sync.dma_start(t[:], gmsg[tt][:])
                gmsg_sb.append(t)
            bw_sb = []
            for i in range(3):
                t = cpool.tile([48 if i == 0 else 96, 96], dt.bfloat16,
                               tag=f"bw{i}", name=f"bw_sb{i}")
                nc.sync.dma_start(t[:], bw[i][:])
                bw_sb.append(t)
            i96_sb = cpool.tile([96, 96], dt.bfloat16)
            nc.sync.dma_start(i96_sb[:], i96[:])
            sel_sb = cpool.tile([96, 6], dt.float32)
            nc.sync.dma_start(sel_sb[:], sel[:])
            selT_sb = cpool.tile([6, 96], dt.float32)
            nc.sync.dma_start(selT_sb[:], selT[:])
            gam_sb, bet_sb = [], []
            for i in range(3):
                g_t = cpool.tile([6, 1], dt.float32, tag=f"gam{i}",
                                 name=f"gam_sb{i}")
                nc.sync.dma_start(g_t[:], gam[i][:])
                gam_sb.append(g_t)
                b_t = cpool.tile([6, 1], dt.float32, tag=f"bet{i}",
                                 name=f"bet_sb{i}")
                nc.sync.dma_start(b_t[:], bet[i][:])
                bet_sb.append(b_t)
            eps_sb = cpool.tile([6, 1], dt.float32, name="eps_sb")
            nc.vector.memset(eps_sb[:], EPS)
            dis_sb = cpool.tile([96, NSP], dt.bfloat16)
            nc.sync.dma_start(dis_sb[:], disb[:])

            # x feature-major, already dis-scaled on host
            h_t = [ho_pool.tile([48, NSP], dt.bfloat16, tag=f"ho{u}",
                                name=f"x_sb{u}") for u in range(2)]
            nc.sync.dma_start(h_t[0][:], x0[:])
            nc.sync.dma_start(h_t[1][:], x1[:])

            lw1v = lw1p.ap().rearrange("f (c p) e -> (f c) p e", p=128)

            def stg_load(si):
                nch = min(16, H * NBLK - si * 16)
                t = stg_pool.tile([128, nch, 256], dt.bfloat16, tag="stg",
                                  name=f"stg{si}")
                nc.scalar.dma_start(
                    t[:], lw1v[si * 16:si * 16 + nch]
                    .rearrange("c p e -> p c e"))
                return t

            # ================= conv layers =================
            for L in range(3):
                kin = 48 if L == 0 else 96
                m_nm = mnm_pool.tile([128, NBLK, EW], dt.bfloat16, tag="mnm",
                                     name=f"mnm_L{L}")
                if L == 0:
                    nc.vector.memset(m_nm[:, :, W:EW], 0.0)

                def emit_mnm(c0, c1, L=L, m_nm=m_nm, h_t=h_t, kin=kin):
                    for u in range(2):
                        for b0 in range(c0, c1, 5):
                            nb = min(5, c1 - b0)
                            pt = ps1.tile([128, 512], dt.float32, tag="ps1",
                                          name=f"ptb_L{L}")
                            for i in range(nb):
                                c = b0 + i
                                nc.tensor.matmul(
                                    pt[:, 96 * i:96 * (i + 1)],
                                    h_t[u][:, 128 * c:128 * (c + 1)],
                                    bw_sb[L][:], start=True, stop=True)
                            src = (pt[:, :96 * nb]
                                   .rearrange("p (c f) -> p c f", f=96))
                            nc.vector.tensor_copy(
                                m_nm[:, b0:b0 + nb, 96 * u:96 * (u + 1)], src)

                # half A -> AllGather0, half B -> AllGather1
                emit_mnm(0, H0C)
                nc.sync.dma_start(
                    m_hbm[0].ap().rearrange("(c p) e -> p c e", p=128),
                    m_nm[:, 0:H0C, :])
                nc.gpsimd.collective_compute(
                    "AllGather", ALU.bypass, replica_groups=groups,
                    ins=[m_hbm[0].ap()], outs=[m_full[0].ap()])
                emit_mnm(H0C, NBLK)
                nc.sync.dma_start(
                    m_hbm[1].ap().rearrange("(c p) e -> p c e", p=128),
                    m_nm[:, H0C:NBLK, :])
                nc.gpsimd.collective_compute(
                    "AllGather", ALU.bypass, replica_groups=groups,
                    ins=[m_hbm[1].ap()], outs=[m_full[1].ap()])

                # m_fm (feature-major self term) runs under the AllGathers
                m_fm = [mfm_pool.tile([96, NSP], dt.bfloat16, tag=f"mfm{u}",
                                      name=f"mfm_L{L}_{u}") for u in range(2)]
                for u in range(2):
                    for c0 in range(0, NSP, 512):
                        cw = min(512, NSP - c0)
                        pt = ps1.tile([128, 512], dt.float32, tag="ps1",
                                      name=f"ptf_L{L}")
                        nc.tensor.matmul(pt[0:96, :cw], bw_sb[L][:],
                                         h_t[u][:, c0:c0 + cw],
                                         start=True, stop=True)
                        nc.vector.tensor_copy(m_fm[u][:, c0:c0 + cw],
                                              pt[0:96, :cw])

                # prefetch lw1 staging during L3's gather passes
                stg_tiles = {}
                if L == 2:
                    for si in range(STG_BUFS):
                        stg_tiles[si] = stg_load(si)

                # gather streams (tokens land in SBUF tiles, 4 queues)
                msg_tiles = [[], []]
                for tt in (0, 1):
                    for ci in range(NCALL[tt]):
                        nch = min(CPC, NCHUNK[tt] - ci * CPC)
                        t = msg_pool.tile([128, nch, EW], dt.bfloat16,
                                          tag="msg", name=f"msg_L{L}_{tt}_{ci}")
                        nc.gpsimd.dma_gather(
                            t[:], m_full[tt].ap(),
                            gmsg_sb[tt][:, ci * (CALL // 16):
                                        ci * (CALL // 16) + nch * 8],
                            num_idxs=nch * 128, num_idxs_reg=nch * 128,
                            elem_size=EW, queue_num=(ci + tt * NCALL[0]) % NQ)
                        msg_tiles[tt].append(t)
                oh_tiles = [[], []]
                for tt in (0, 1):
                    for ti in range(NOHT[tt]):
                        nch = min(16, NCHUNK[tt] - ti * 16)
                        t = oh_pool.tile([128, nch, 128], dt.float8e4,
                                         tag="oh", name=f"oh_L{L}_{tt}_{ti}")
                        nc.sync.dma_start(
                            t[:], oh_in[tt][:, ti * 16:ti * 16 + nch, :])
                        oh_tiles[tt].append(t)

                o_acc = [oacc_pool.tile([96, NSP], dt.bfloat16, tag=f"oa{u}",
                                        name=f"oacc_L{L}_{u}")
                         for u in range(2)]
                o_t = [ho_pool.tile([96, NSP], dt.bfloat16, tag=f"ho{u}",
                                    name=f"o_L{L}_{u}") for u in range(2)]
                S_t = st_pool.tile([96, 4 * NW], dt.float32, tag="S")

                def seg_window(tt, w, L=L, msg_tiles=msg_tiles,
                               oh_tiles=oh_tiles, o_acc=o_acc, o_t=o_t,
                               m_fm=m_fm, S_t=S_t):
                    jlo = w * BLK_PER_W
                    jhi = min(jlo + BLK_PER_W, NBLK)
                    pw = [psA.tile([96, 512], dt.float32, tag="psA",
                                   name=f"pw_L{L}_{tt}_{w}_{uu}")
                          for uu in range(2)]
                    for j in range(jlo, jhi):
                        nch_b = int(CH[tt][j])
                        for ci in range(nch_b):
                            q = int(CHOFF[tt][j]) + ci
                            mt = msg_tiles[tt][q // CPC]
                            ot = oh_tiles[tt][q // 16]
                            for u in range(2):
                                nc.tensor.matmul(
                                    pw[u][:, 128 * (j - jlo):
                                          128 * (j - jlo + 1)],
                                    mt[:, q % CPC, 96 * u:96 * (u + 1)],
                                    ot[:, q % 16, :],
                                    start=(ci == 0),
                                    stop=(ci == nch_b - 1))
                    c0 = 512 * w
                    cw = min(512, NS - c0)    # stats over real nodes only
                    cwf = min(512, NSP - c0)
                    for u in range(2):
                        if tt == 0:
                            # evict pass-A partial + self term
                            nc.vector.tensor_add(
                                o_acc[u][:, c0:c0 + cwf],
                                pw[u][:, :cwf], m_fm[u][:, c0:c0 + cwf])
                        else:
                            y = y_pool.tile([96, 512], dt.float32, tag="y")
                            nc.vector.tensor_add(
                                y[:, :cwf], pw[u][:, :cwf],
                                o_acc[u][:, c0:c0 + cwf])
                            nc.vector.tensor_mul(
                                y[:, :cwf], y[:, :cwf],
                                dis_sb[:, c0:c0 + cwf])
                            nc.vector.tensor_reduce(
                                S_t[:, 2 * w + u:2 * w + u + 1],
                                y[:, :cw], axis=mybir.AxisListType.X,
                                op=ALU.add)
                            y2 = y_pool.tile([96, 512], dt.float32, tag="y")
                            acc = acc_pool.tile([96, 1], dt.float32,
                                                tag="acc",
                                                name=f"acc_{L}_{w}_{u}")
                            nc.scalar.activation(
                                y2[:, :cw], y[:, :cw], AF.Square,
                                accum_out=acc[:])
                            nc.vector.tensor_copy(
                                S_t[:, 2 * (NW + w) + u:
                                    2 * (NW + w) + u + 1], acc[:])
                            nc.vector.tensor_copy(
                                o_t[u][:, c0:c0 + cwf], y[:, :cwf])

                for tt in (0, 1):      # pass A (table 0), pass B (table 1)
                    for w in range(NW):
                        seg_window(tt, w)

                # ---------- BN stats + apply ----------
                st2 = st_pool.tile([96, 4], dt.float32, tag="st2")
                for u in range(2):
                    nc.vector.tensor_reduce(
                        st2[:, u:u + 1],
                        S_t[:, :2 * NW].rearrange("p (w u) -> p u w", u=2)
                        [:, u, :], axis=mybir.AxisListType.X, op=ALU.add)
                    nc.vector.tensor_reduce(
                        st2[:, 2 + u:3 + u],
                        S_t[:, 2 * NW:4 * NW]
                        .rearrange("p (w u) -> p u w", u=2)[:, u, :],
                        axis=mybir.AxisListType.X, op=ALU.add)
                pst = psD.tile([6, 2], dt.float32, tag="pst")
                for u in range(2):
                    nc.tensor.matmul(
                        pst[:], sel_sb[:],
                        st2[:, :].rearrange("p (a u) -> p u a", u=2)[:, u, :],
                        start=(u == 0), stop=(u == 1))
                stt = t6_pool.tile([6, 2], dt.float32, tag="stt")
                nc.vector.tensor_copy(stt[:], pst[:])
                nc.sync.dma_start(st_in[L].ap(), stt[:])
                nc.gpsimd.collective_compute(
                    "AllReduce", ALU.add, replica_groups=groups,
                    ins=[st_in[L].ap()], outs=[st_out[L].ap()])
                sto = t6_pool.tile([6, 2], dt.float32, tag="sto")
                nc.sync.dma_start(sto[:], st_out[L].ap())
                mu = t6_pool.tile([6, 1], dt.float32, tag="mu")
                nc.vector.tensor_scalar_mul(mu[:], sto[:, 0:1], 1.0 / NB)
                var = t6_pool.tile([6, 1], dt.float32, tag="var")
                nc.vector.tensor_scalar_mul(var[:], sto[:, 1:2], 1.0 / NB)
                musq = t6_pool.tile([6, 1], dt.float32, tag="musq")
                nc.vector.tensor_mul(musq[:], mu[:], mu[:])
                nc.vector.tensor_sub(var[:], var[:], musq[:])
                nc.scalar.activation(var[:], var[:], AF.Sqrt,
                                     bias=eps_sb[:].opt())
                nc.vector.reciprocal(var[:], var[:])     # var := 1/sigma
                ab6 = t6_pool.tile([6, 2], dt.float32, tag="ab6")
                nc.vector.tensor_mul(ab6[:, 0:1], gam_sb[L][:], var[:])
                nc.vector.tensor_mul(musq[:], mu[:], ab6[:, 0:1])
                nc.vector.tensor_sub(ab6[:, 1:2], bet_sb[L][:], musq[:])
                pab = psD.tile([96, 2], dt.float32, tag="pab")
                nc.tensor.matmul(pab[:], selT_sb[:], ab6[:],
                                 start=True, stop=True)
                ab = st_pool.tile([96, 2], dt.float32, tag="ab")
                nc.vector.tensor_copy(ab[:], pab[:])
                # BN apply + relu (in place); fold dis for the next conv
                for u in range(2):
                    nc.scalar.activation(o_t[u][:], o_t[u][:], AF.Relu,
                                         bias=ab[:, 1:2].opt(),
                                         scale=ab[:, 0:1].opt())
                    if L < 2:
                        nc.vector.tensor_mul(o_t[u][:], o_t[u][:], dis_sb[:])
                h_t = [o_t[0], o_t[1]]

            # ================= MLP head =================
            # o -> node-major bf16 (transpose via PE with identity)
            o_bf = mnm_pool.tile([128, NBLK, EW], dt.bfloat16, tag="mnm",
                                 name="o_bf")
            for u in range(2):
                for b0 in range(0, NBLK, 5):
                    nb = min(5, NBLK - b0)
                    pt = ps1.tile([128, 512], dt.float32, tag="ps1",
                                  name="pto")
                    for i in range(nb):
                        c = b0 + i
                        nc.tensor.matmul(pt[:, 96 * i:96 * (i + 1)],
                                         h_t[u][:, 128 * c:128 * (c + 1)],
                                         i96_sb[:], start=True, stop=True)
                    src = pt[:, :96 * nb].rearrange("p (c f) -> p c f", f=96)
                    nc.vector.tensor_copy(
                        o_bf[:, b0:b0 + nb, 96 * u:96 * (u + 1)], src)

            zt = [psD.tile([128, 32], dt.float32, tag="pst", name="zt0"),
                  psD.tile([128, 32], dt.float32, tag="pab", name="zt1")]
            for si in range(STG_BUFS, NSTG):
                stg_tiles[si] = stg_load(si)
            for f in range(H):
                for c in range(NBLK):
                    fc = f * NBLK + c
                    st_t = stg_tiles[fc // 16]
                    rhs = (o_bf[:, c, 0:W]
                           .rearrange("p (g f) -> p f g", f=H)[:, f, :])
                    for h2 in range(2):
                        nc.tensor.matmul(
                            zt[h2][:],
                            st_t[:, fc % 16, 128 * h2:128 * (h2 + 1)],
                            rhs, start=(fc == 0), stop=(fc == H * NBLK - 1))
            zc = [st_pool.tile([128, 32], dt.float32, tag=f"zc{h2}",
                               name=f"zc{h2}") for h2 in range(2)]
            for h2 in range(2):
                nc.vector.tensor_copy(zc[h2][:], zt[h2][:])
                nc.sync.dma_start(mlp_in.ap()[h2], zc[h2][:])
            nc.gpsimd.collective_compute(
                "AllReduce", ALU.add, replica_groups=groups,
                ins=[mlp_in.ap()], outs=[mlp_out.ap()])
            lb1_sb = cpool.tile([128, 2], dt.float32)
            nc.sync.dma_start(lb1_sb[:], lb1h[:])
            lw2_sb = cpool.tile([128, 2, 128], dt.float32)
            nc.sync.dma_start(lw2_sb[:], lw2r[:])
            lw3_sb = cpool.tile([128, 64], dt.float32)
            nc.sync.dma_start(lw3_sb[:], lw3[:])
            lw4_sb = cpool.tile([64, 32], dt.float32)
            nc.sync.dma_start(lw4_sb[:], lw4[:])
            lb2_sb = cpool.tile([128, 1], dt.float32)
            nc.sync.dma_start(lb2_sb[:], lb2c[:])
            lb3_sb = cpool.tile([64, 1], dt.float32)
            nc.sync.dma_start(lb3_sb[:], lb3c[:])
            lb4_sb = cpool.tile([32, 1], dt.float32)
            nc.sync.dma_start(lb4_sb[:], lb4c[:])

            h1 = [st_pool.tile([128, 32], dt.float32, tag=f"h1{h2}",
                               name=f"h1_{h2}") for h2 in range(2)]
            for h2 in range(2):
                nc.sync.dma_start(h1[h2][:], mlp_out.ap()[h2])
                nc.scalar.activation(h1[h2][:], h1[h2][:], AF.Relu,
                                     bias=lb1_sb[:, h2:h2 + 1].opt())
            p2 = psD.tile([128, 32], dt.float32, tag="pst")
            for h2 in range(2):
                nc.tensor.matmul(p2[:], lw2_sb[:, h2, :], h1[h2][:],
                                 start=(h2 == 0), stop=(h2 == 1))
            h2x = st_pool.tile([128, 32], dt.float32, tag="h2x")
            nc.scalar.activation(h2x[:], p2[:], AF.Relu, bias=lb2_sb[:].opt())
            p3 = psD.tile([64, 32], dt.float32, tag="pab")
            nc.tensor.matmul(p3[:], lw3_sb[:], h2x[:], start=True, stop=True)
            h3 = st_pool.tile([64, 32], dt.float32, tag="h3")
            nc.scalar.activation(h3[:], p3[:], AF.Relu, bias=lb3_sb[:].opt())
            p4 = psD.tile([32, 32], dt.float32, tag="pst")
            nc.tensor.matmul(p4[:], lw4_sb[:], h3[:], start=True, stop=True)
            o4 = st_pool.tile([32, 32], dt.float32, tag="o4")
            nc.vector.tensor_scalar_add(o4[:], p4[:], lb4_sb[:].opt())
            nc.sync.dma_start(out_d.ap().rearrange("g c -> c g"), o4[:])

    nc.compile()
    return nc


def _make_in_maps(plan, inputs):
    f32 = np.float32
    bf16 = ml_dtypes.bfloat16
    x = np.asarray(inputs["x"], f32)
    W1 = np.asarray(inputs["W1"], f32)
    W2 = np.asarray(inputs["W2"], f32)
    W3 = np.asarray(inputs["W3"], f32)
    lw1 = np.asarray(inputs["lw1"], f32)
    lw2 = np.asarray(inputs["lw2"], f32)
    lw3 = np.asarray(inputs["lw3"], f32)
    lw4 = np.asarray(inputs["lw4"], f32)

    bw1 = np.kron(np.eye(16, dtype=f32), W1).astype(bf16)   # [48, 96]
    bw2 = np.kron(np.eye(16, dtype=f32), W2).astype(bf16)   # [96, 96]
    bw3 = np.kron(np.eye(16, dtype=f32), W3).astype(bf16)
    i96 = np.eye(96, dtype=f32).astype(bf16)
    sel = np.tile(np.eye(6, dtype=f32), (16, 1))             # [96, 6]
    selT = np.ascontiguousarray(sel.T)                       # [6, 96]
    lw2r = np.ascontiguousarray(
        lw2.reshape(2, 128, 128).transpose(1, 0, 2))         # [128, 2, 128]
    lb1h = np.ascontiguousarray(
        np.asarray(inputs["lb1"], f32).reshape(2, 128).T)    # [128, 2]

    dis = plan["dis"]                                        # [N]
    xg = x.reshape(B, N, DIM)

    common = {
        "bw1": bw1, "bw2": bw2, "bw3": bw3, "i96": i96,
        "sel": sel, "selT": selT,
        "g1": np.asarray(inputs["g1"], f32).reshape(6, 1),
        "be1": np.asarray(inputs["be1"], f32).reshape(6, 1),
        "g2": np.asarray(inputs["g2"], f32).reshape(6, 1),
        "be2": np.asarray(inputs["be2"], f32).reshape(6, 1),
        "g3": np.asarray(inputs["g3"], f32).reshape(6, 1),
        "be3": np.asarray(inputs["be3"], f32).reshape(6, 1),
        "lw2r": lw2r, "lw3": lw3, "lw4": lw4,
        "lb1h": lb1h,
        "lb2c": np.asarray(inputs["lb2"], f32).reshape(128, 1),
        "lb3c": np.asarray(inputs["lb3"], f32).reshape(64, 1),
        "lb4c": np.asarray(inputs["lb4"], f32).reshape(32, 1),
    }

    in_maps = []
    for k in range(NC):
        pc = plan["per_core"][k]
        dk = dis[NS * k:NS * (k + 1)]
        # x feature-major, dis-folded: xq[u][g*3+fi, n]
        xs = xg[:, NS * k:NS * (k + 1), :]              # [32, 6250, 3]
        xq = np.zeros((2, 48, NSP), f32)
        for u in range(2):
            blkv = (xs[16 * u:16 * (u + 1)] * dk[None, :, None])
            xq[u, :, :NS] = blkv.transpose(0, 2, 1).reshape(48, NS)
        disb = np.ones((96, NSP), f32)
        disb[:, :NS] = dk[None, :]
        # lw1 rows f-major per core, zero-padded nodes
        lw1s = lw1[NS * k * H:NS * (k + 1) * H].reshape(NS, H, 256)
        lw1p = np.zeros((H, NSP, 256), bf16)
        lw1p[:, :NS, :] = lw1s.transpose(1, 0, 2).astype(bf16)
        m = dict(common)
        m.update({
            "x0": np.ascontiguousarray(xq[0]).astype(bf16),
            "x1": np.ascontiguousarray(xq[1]).astype(bf16),
            "disb": disb.astype(bf16),
            "gmsg0": pc["gmsg0"], "gmsg1": pc["gmsg1"],
            "oh0": pc["oh0"], "oh1": pc["oh1"],
            "lw1p": lw1p,
        })
        in_maps.append(m)
    return in_maps


def _get(edge_base):
    key = hash(np.asarray(edge_base).tobytes())
    if key not in _cache:
        plan = _build_plan(np.asarray(edge_base))
        nc = _build_nc(plan)
        _cache[key] = (plan, nc)
    return _cache[key]


def kernel(**inputs):
    from concourse.bass_utils import run_bass_kernel_spmd
    assert int(inputs["num_graphs"]) == B and int(inputs["num_nodes"]) == N
    plan, nc = _get(inputs["edge_base"])
    in_maps = _make_in_maps(plan, inputs)
    trace = os.environ.get("KERNEL_TRACE", "0") == "1"
    res = run_bass_kernel_spmd(nc, in_maps, core_ids=list(range(NC)),
                               trace=trace)
    kernel.last_result = res
    return np.ascontiguousarray(res.results[0]["out"])


# revision 11
# speedup vs baseline: 1.2610x; 1.0193x over previous
"""Trainium2 Bass kernel for nn_Encoder (3-layer GCN + BatchNorm + MLP head).

Sharding: nodes sharded across 8 cores (6250-node slices, all 32 graphs at
full 192-float width).  Per layer:
  - transform runs in bf16 on the PE; the node-major message table is split
    into two half-tables (3200 + 3072 rows/core) that are AllGather'd
    separately so the second AllGather and the first gather/segment-sum pass
    overlap
  - messages are fetched straight from the gathered half-tables with
    dma_gather (512B rows, int16 indices fit because each half-table has
    <32768 rows) over 4 SWDGE queues; no intermediate compaction
  - the segment-sum is a PE matmul against host-built fp8 one-hot matrices;
    pass A (table-0 tokens) evicts partial sums (+self term) to SBUF, pass B
    (table-1 tokens) finishes, scales by dis, and computes BN stats
  - BN statistics are pooled with a 12-float AllReduce
The MLP head contracts the 300000-dim axis with lw1 row-sharded per core
(bf16, streamed through SBUF with prefetch) and a [256,32] AllReduce.
"""
import os
import numpy as np
import ml_dtypes

N = 50000
B = 32
E = 150000
DIM = 3
H = 6
NC = 8
NS = N // NC            # 6250 nodes per core
NSP = 6272              # padded to 49*128
NBLK = NSP // 128       # 49 dst chunks
H0C = 25                # chunks in half-table 0
H1C = NBLK - H0C        # 24
H0 = H0C * 128          # 3200 rows/core in table 0
H1 = H1C * 128          # 3072 rows/core in table 1
EW = 256                # padded bf16 row width (512 bytes)
W = B * H               # 192 payload floats per row
NB = B * N
EPS = 1e-5
CALL = 1024             # gather tokens per call (SWDGE ring limit)
CPC = CALL // 128       # msg chunks per call
NQ = 4                  # SWDGE queues
BLK_PER_W = 4           # dst chunks per psum window
NW = (NBLK + BLK_PER_W - 1) // BLK_PER_W   # 13 windows

_cache = {}


def _wrap_idx(arr):
    """[n] int array -> [128, n/16] int16 device layout (16-wrap, replicated
    for the 8 Q7 cores)."""
    n = len(arr)
    assert n % 16 == 0
    w = arr.reshape(n // 16, 16).T.astype(np.int16)
    return np.ascontiguousarray(np.tile(w, (8, 1)))


def _build_plan(edge_base):
    """Host-side index preprocessing. Returns uniform shapes + per-core data."""
    row = np.asarray(edge_base[0], dtype=np.int64)
    col = np.asarray(edge_base[1], dtype=np.int64)
    deg = (np.bincount(col, minlength=N) + 1).astype(np.float32)
    dis = 1.0 / np.sqrt(deg)

    # per-core edge lists, split by source half-table
    cores = []
    cnt = np.zeros((2, NC, NBLK), np.int64)
    for k in range(NC):
        sel = (col // NS) == k
        src = row[sel]
        dstl = col[sel] - NS * k
        j = src // NS
        off = src - j * NS
        t = (off >= H0).astype(np.int64)
        trow = np.where(t == 0, j * H0 + off, j * H1 + (off - H0))
        b = dstl // 128
        cores.append((t, b, trow, dstl))
        for tt in (0, 1):
            cnt[tt, k] = np.bincount(b[t == tt], minlength=NBLK)

    # uniform (cross-core) chunk counts per (block, table)
    CH = [np.maximum(1, -(-cnt[tt].max(axis=0) // 128)).astype(int)
          for tt in (0, 1)]
    choff = [np.concatenate([[0], np.cumsum(CH[tt])]) for tt in (0, 1)]
    nchunk = [int(CH[tt].sum()) for tt in (0, 1)]
    tok = [nchunk[tt] * 128 for tt in (0, 1)]

    per_core = []
    for k in range(NC):
        t, b, trow, dstl = cores[k]
        gmsg = [np.zeros(tok[tt], np.int64) for tt in (0, 1)]
        oh = [np.zeros((tok[tt], 128), np.float32) for tt in (0, 1)]
        for tt in (0, 1):
            m = t == tt
            bb, rr, dd = b[m], trow[m], dstl[m]
            order = np.argsort(bb, kind="stable")
            bb, rr, dd = bb[order], rr[order], dd[order]
            starts = np.searchsorted(bb, np.arange(NBLK))
            ends = np.searchsorted(bb, np.arange(NBLK) + 1)
            for jb in range(NBLK):
                s, e = int(starts[jb]), int(ends[jb])
                n = e - s
                base = int(choff[tt][jb]) * 128
                assert n <= CH[tt][jb] * 128
                gmsg[tt][base:base + n] = rr[s:e]
                oh[tt][base + np.arange(n), dd[s:e] - 128 * jb] = 1.0
        pc = {}
        for tt in (0, 1):
            oh_dev = (oh[tt].reshape(nchunk[tt], 128, 128)
                      .transpose(1, 0, 2))
            pc[f"gmsg{tt}"] = _wrap_idx(gmsg[tt])
            pc[f"oh{tt}"] = np.ascontiguousarray(
                oh_dev.astype(ml_dtypes.float8_e4m3))
        per_core.append(pc)

    return {
        "dis": dis, "CH": CH, "choff": choff, "nchunk": nchunk, "tok": tok,
        "per_core": per_core,
    }


def _build_nc(plan):
    import concourse.bacc as bacc
    import concourse.mybir as mybir
    import concourse.tile as tile

    dt = mybir.dt
    AF = mybir.ActivationFunctionType
    ALU = mybir.AluOpType
    CH = plan["CH"]
    CHOFF = plan["choff"]
    NCHUNK = plan["nchunk"]
    TOK = plan["tok"]
    NCALL = [-(-TOK[tt] // CALL) for tt in (0, 1)]
    NOHT = [-(-NCHUNK[tt] // 16) for tt in (0, 1)]

    nc = bacc.Bacc("TRN2", target_bir_lowering=False, debug=False,
                   num_devices=NC, enable_asserts=False, num_swdge_queues=NQ)

    # ---------------- I/O ----------------
    def inp(name, shape, d):
        return nc.dram_tensor(name, shape, d, kind="ExternalInput")

    x0 = inp("x0", [48, NSP], dt.bfloat16)
    x1 = inp("x1", [48, NSP], dt.bfloat16)
    disb = inp("disb", [96, NSP], dt.bfloat16)
    gmsg = [inp(f"gmsg{tt}", [128, TOK[tt] // 16], dt.int16) for tt in (0, 1)]
    oh_in = [inp(f"oh{tt}", [128, NCHUNK[tt], 128], dt.float8e4)
             for tt in (0, 1)]
    bw = [inp("bw1", [48, 96], dt.bfloat16),
          inp("bw2", [96, 96], dt.bfloat16),
          inp("bw3", [96, 96], dt.bfloat16)]
    i96 = inp("i96", [96, 96], dt.bfloat16)
    sel = inp("sel", [96, 6], dt.float32)
    selT = inp("selT", [6, 96], dt.float32)
    gam = [inp(f"g{i}", [6, 1], dt.float32) for i in (1, 2, 3)]
    bet = [inp(f"be{i}", [6, 1], dt.float32) for i in (1, 2, 3)]
    lw1q = inp("lw1q", [128, H * NBLK, 256], dt.bfloat16)
    lw2r = inp("lw2r", [128, 2, 128], dt.float32)
    lw3 = inp("lw3", [128, 64], dt.float32)
    lw4 = inp("lw4", [64, 32], dt.float32)
    lb1h = inp("lb1h", [128, 2], dt.float32)
    lb2c = inp("lb2c", [128, 1], dt.float32)
    lb3c = inp("lb3c", [64, 1], dt.float32)
    lb4c = inp("lb4c", [32, 1], dt.float32)
    out_d = nc.dram_tensor("out", [B, 32], dt.float32, kind="ExternalOutput")

    m_hbm = [nc.dram_tensor("m_hbm0", [H0, EW], dt.bfloat16, kind="Internal"),
             nc.dram_tensor("m_hbm1", [H1, EW], dt.bfloat16, kind="Internal")]
    m_full = [nc.dram_tensor("m_full0", [NC * H0, EW], dt.bfloat16,
                             kind="Internal", addr_space="Shared"),
              nc.dram_tensor("m_full1", [NC * H1, EW], dt.bfloat16,
                             kind="Internal", addr_space="Shared")]
    st_in = [nc.dram_tensor(f"st_in{i}", [6, 2], dt.float32, kind="Internal")
             for i in range(3)]
    st_out = [nc.dram_tensor(f"st_out{i}", [6, 2], dt.float32, kind="Internal",
                             addr_space="Shared") for i in range(3)]
    mlp_in = nc.dram_tensor("mlp_in", [2, 128, 32], dt.float32, kind="Internal")
    mlp_out = nc.dram_tensor("mlp_out", [2, 128, 32], dt.float32,
                             kind="Internal", addr_space="Shared")

    groups = [list(range(NC))]
    NSTG = (H * NBLK + 15) // 16       # lw1 staging tiles
    STG_BUFS = 5

    with tile.TileContext(nc) as tc:
        with (
            tc.tile_pool(name="const", bufs=1) as cpool,
            tc.tile_pool(name="ho", bufs=1) as ho_pool,
            tc.tile_pool(name="mfm", bufs=1) as mfm_pool,
            tc.tile_pool(name="mnm", bufs=1) as mnm_pool,
            tc.tile_pool(name="msg", bufs=10) as msg_pool,
            tc.tile_pool(name="ohp", bufs=4) as oh_pool,
            tc.tile_pool(name="ysc", bufs=2) as y_pool,
            tc.tile_pool(name="st", bufs=1) as st_pool,
            tc.tile_pool(name="t6", bufs=1) as t6_pool,
            tc.tile_pool(name="stg", bufs=STG_BUFS) as stg_pool,
            tc.tile_pool(name="psA", bufs=4, space="PSUM") as psA,
            tc.tile_pool(name="ps1", bufs=2, space="PSUM") as ps1,
            tc.tile_pool(name="psD", bufs=1, space="PSUM") as psD,
        ):
            # ---------- setup ----------
            gmsg_sb = []
            for tt in (0, 1):
                t = cpool.tile([128, TOK[tt] // 16], dt.int16,
                               tag=f"gmsg{tt}", name=f"gmsg_sb{tt}")
                nc.sync.dma_start(t[:], gmsg[tt][:])
                gmsg_sb.append(t)
            bw_sb = []
            for i in range(3):
                t = cpool.tile([48 if i == 0 else 96, 96], dt.bfloat16,
                               tag=f"bw{i}", name=f"bw_sb{i}")
                nc.sync.dma_start(t[:], bw[i][:])
                bw_sb.append(t)
            i96_sb = cpool.tile([96, 96], dt.bfloat16)
            nc.sync.dma_start(i96_sb[:], i96[:])
            sel_sb = cpool.tile([96, 6], dt.float32)
            nc.sync.dma_start(sel_sb[:], sel[:])
            selT_sb = cpool.tile([6, 96], dt.float32)
            nc.sync.dma_start(selT_sb[:], selT[:])
            gam_sb, bet_sb = [], []
            for i in range(3):
                g_t = cpool.tile([6, 1], dt.float32, tag=f"gam{i}",
                                 name=f"gam_sb{i}")
                nc.sync.dma_start(g_t[:], gam[i][:])
                gam_sb.append(g_t)
                b_t = cpool.tile([6, 1], dt.float32, tag=f"bet{i}",
                                 name=f"bet_sb{i}")
                nc.sync.dma_start(b_t[:], bet[i][:])
                bet_sb.append(b_t)
            eps_sb = cpool.tile([6, 1], dt.float32, name="eps_sb")
            nc.vector.memset(eps_sb[:], EPS)
            dis_sb = cpool.tile([96, NSP], dt.bfloat16)
            nc.sync.dma_start(dis_sb[:], disb[:])

            # x feature-major, already dis-scaled on host
            h_t = [ho_pool.tile([48, NSP], dt.bfloat16, tag=f"ho{u}",
                                name=f"x_sb{u}") for u in range(2)]
            nc.sync.dma_start(h_t[0][:], x0[:])
            nc.sync.dma_start(h_t[1][:], x1[:])

            def stg_load(si):
                nch = min(16, H * NBLK - si * 16)
                t = stg_pool.tile([128, nch, 256], dt.bfloat16, tag="stg",
                                  name=f"stg{si}")
                nc.scalar.dma_start(t[:], lw1q[:, si * 16:si * 16 + nch, :])
                return t

            # ================= conv layers =================
            for L in range(3):
                kin = 48 if L == 0 else 96
                m_nm = mnm_pool.tile([128, NBLK, EW], dt.bfloat16, tag="mnm",
                                     name=f"mnm_L{L}")
                if L == 0:
                    nc.vector.memset(m_nm[:, :, W:EW], 0.0)

                def emit_mnm(c0, c1, L=L, m_nm=m_nm, h_t=h_t, kin=kin):
                    for u in range(2):
                        for b0 in range(c0, c1, 5):
                            nb = min(5, c1 - b0)
                            pt = ps1.tile([128, 512], dt.float32, tag="ps1",
                                          name=f"ptb_L{L}")
                            for i in range(nb):
                                c = b0 + i
                                nc.tensor.matmul(
                                    pt[:, 96 * i:96 * (i + 1)],
                                    h_t[u][:, 128 * c:128 * (c + 1)],
                                    bw_sb[L][:], start=True, stop=True)
                            src = (pt[:, :96 * nb]
                                   .rearrange("p (c f) -> p c f", f=96))
                            nc.vector.tensor_copy(
                                m_nm[:, b0:b0 + nb, 96 * u:96 * (u + 1)], src)

                # half A -> AllGather0, half B -> AllGather1
                emit_mnm(0, H0C)
                nc.sync.dma_start(
                    m_hbm[0].ap().rearrange("(c p) e -> p c e", p=128),
                    m_nm[:, 0:H0C, :])
                nc.gpsimd.collective_compute(
                    "AllGather", ALU.bypass, replica_groups=groups,
                    ins=[m_hbm[0].ap()], outs=[m_full[0].ap()])
                emit_mnm(H0C, NBLK)
                nc.sync.dma_start(
                    m_hbm[1].ap().rearrange("(c p) e -> p c e", p=128),
                    m_nm[:, H0C:NBLK, :])
                nc.gpsimd.collective_compute(
                    "AllGather", ALU.bypass, replica_groups=groups,
                    ins=[m_hbm[1].ap()], outs=[m_full[1].ap()])

                # m_fm (feature-major self term) runs under the AllGathers
                m_fm = [mfm_pool.tile([96, NSP], dt.bfloat16, tag=f"mfm{u}",
                                      name=f"mfm_L{L}_{u}") for u in range(2)]
                for u in range(2):
                    for c0 in range(0, NSP, 512):
                        cw = min(512, NSP - c0)
                        pt = ps1.tile([128, 512], dt.float32, tag="ps1",
                                      name=f"ptf_L{L}")
                        nc.tensor.matmul(pt[0:96, :cw], bw_sb[L][:],
                                         h_t[u][:, c0:c0 + cw],
                                         start=True, stop=True)
                        nc.vector.tensor_copy(m_fm[u][:, c0:c0 + cw],
                                              pt[0:96, :cw])

                # prefetch lw1 staging during L3's gather passes
                stg_tiles = {}
                if L == 2:
                    for si in range(STG_BUFS):
                        stg_tiles[si] = stg_load(si)

                # gather streams (tokens land in SBUF tiles, 4 queues)
                msg_tiles = [[], []]
                for tt in (0, 1):
                    for ci in range(NCALL[tt]):
                        nch = min(CPC, NCHUNK[tt] - ci * CPC)
                        t = msg_pool.tile([128, nch, EW], dt.bfloat16,
                                          tag="msg", name=f"msg_L{L}_{tt}_{ci}")
                        nc.gpsimd.dma_gather(
                            t[:], m_full[tt].ap(),
                            gmsg_sb[tt][:, ci * (CALL // 16):
                                        ci * (CALL // 16) + nch * 8],
                            num_idxs=nch * 128, num_idxs_reg=nch * 128,
                            elem_size=EW, queue_num=(ci + tt * NCALL[0]) % NQ)
                        msg_tiles[tt].append(t)
                oh_tiles = [[], []]
                for tt in (0, 1):
                    for ti in range(NOHT[tt]):
                        nch = min(16, NCHUNK[tt] - ti * 16)
                        t = oh_pool.tile([128, nch, 128], dt.float8e4,
                                         tag="oh", name=f"oh_L{L}_{tt}_{ti}")
                        nc.sync.dma_start(
                            t[:], oh_in[tt][:, ti * 16:ti * 16 + nch, :])
                        oh_tiles[tt].append(t)

                # o_acc reuses the m_nm buffer (free after the m_hbm writes)
                oat = mnm_pool.tile([128, NBLK, EW], dt.bfloat16, tag="mnm",
                                    name=f"oacc_L{L}")
                oav = oat[:].rearrange("p c e -> p (c e)")
                o_acc = [oav[0:96, 0:NSP], oav[0:96, NSP:2 * NSP]]
                o_t = [ho_pool.tile([96, NSP], dt.bfloat16, tag=f"ho{u}",
                                    name=f"o_L{L}_{u}") for u in range(2)]
                S_t = st_pool.tile([96, 4 * NW], dt.float32, tag="S")

                def seg_window(tt, w, L=L, msg_tiles=msg_tiles,
                               oh_tiles=oh_tiles, o_acc=o_acc, o_t=o_t,
                               m_fm=m_fm, S_t=S_t):
                    jlo = w * BLK_PER_W
                    jhi = min(jlo + BLK_PER_W, NBLK)
                    pw = [psA.tile([96, 512], dt.float32, tag="psA",
                                   name=f"pw_L{L}_{tt}_{w}_{uu}")
                          for uu in range(2)]
                    for j in range(jlo, jhi):
                        nch_b = int(CH[tt][j])
                        for ci in range(nch_b):
                            q = int(CHOFF[tt][j]) + ci
                            mt = msg_tiles[tt][q // CPC]
                            ot = oh_tiles[tt][q // 16]
                            for u in range(2):
                                nc.tensor.matmul(
                                    pw[u][:, 128 * (j - jlo):
                                          128 * (j - jlo + 1)],
                                    mt[:, q % CPC, 96 * u:96 * (u + 1)],
                                    ot[:, q % 16, :],
                                    start=(ci == 0),
                                    stop=(ci == nch_b - 1))
                    c0 = 512 * w
                    cw = min(512, NS - c0)    # stats over real nodes only
                    cwf = min(512, NSP - c0)
                    for u in range(2):
                        if tt == 0:
                            # evict pass-A partial + self term
                            nc.vector.tensor_add(
                                o_acc[u][:, c0:c0 + cwf],
                                pw[u][:, :cwf], m_fm[u][:, c0:c0 + cwf])
                        else:
                            y = y_pool.tile([96, 512], dt.float32, tag="y")
                            nc.vector.tensor_add(
                                y[:, :cwf], pw[u][:, :cwf],
                                o_acc[u][:, c0:c0 + cwf])
                            nc.vector.tensor_mul(
                                o_t[u][:, c0:c0 + cwf], y[:, :cwf],
                                dis_sb[:, c0:c0 + cwf])
                            # stats: sum + sum-of-squares via Act accumulators
                            y2 = y_pool.tile([96, 512], dt.bfloat16,
                                             tag="y2")
                            nc.scalar.activation(
                                y2[:, :cw], o_t[u][:, c0:c0 + cw], AF.Copy,
                                accum_out=S_t[:, 2 * w + u:2 * w + u + 1])
                            nc.scalar.activation(
                                y2[:, :cw], o_t[u][:, c0:c0 + cw], AF.Square,
                                accum_out=S_t[:, 2 * (NW + w) + u:
                                              2 * (NW + w) + u + 1])

                for tt in (0, 1):      # pass A (table 0), pass B (table 1)
                    for w in range(NW):
                        seg_window(tt, w)

                # ---------- BN stats + apply ----------
                st2 = st_pool.tile([96, 4], dt.float32, tag="st2")
                for u in range(2):
                    nc.vector.tensor_reduce(
                        st2[:, u:u + 1],
                        S_t[:, :2 * NW].rearrange("p (w u) -> p u w", u=2)
                        [:, u, :], axis=mybir.AxisListType.X, op=ALU.add)
                    nc.vector.tensor_reduce(
                        st2[:, 2 + u:3 + u],
                        S_t[:, 2 * NW:4 * NW]
                        .rearrange("p (w u) -> p u w", u=2)[:, u, :],
                        axis=mybir.AxisListType.X, op=ALU.add)
                pst = psD.tile([6, 2], dt.float32, tag="pst")
                for u in range(2):
                    nc.tensor.matmul(
                        pst[:], sel_sb[:],
                        st2[:, :].rearrange("p (a u) -> p u a", u=2)[:, u, :],
                        start=(u == 0), stop=(u == 1))
                stt = t6_pool.tile([6, 2], dt.float32, tag="stt")
                nc.vector.tensor_copy(stt[:], pst[:])
                nc.sync.dma_start(st_in[L].ap(), stt[:])
                nc.gpsimd.collective_compute(
                    "AllReduce", ALU.add, replica_groups=groups,
                    ins=[st_in[L].ap()], outs=[st_out[L].ap()])
                sto = t6_pool.tile([6, 2], dt.float32, tag="sto")
                nc.sync.dma_start(sto[:], st_out[L].ap())
                mu = t6_pool.tile([6, 1], dt.float32, tag="mu")
                nc.vector.tensor_scalar_mul(mu[:], sto[:, 0:1], 1.0 / NB)
                var = t6_pool.tile([6, 1], dt.float32, tag="var")
                nc.vector.tensor_scalar_mul(var[:], sto[:, 1:2], 1.0 / NB)
                musq = t6_pool.tile([6, 1], dt.float32, tag="musq")
                nc.vector.tensor_mul(musq[:], mu[:], mu[:])
                nc.vector.tensor_sub(var[:], var[:], musq[:])
                nc.scalar.activation(var[:], var[:], AF.Sqrt,
                                     bias=eps_sb[:].opt())
                nc.vector.reciprocal(var[:], var[:])     # var := 1/sigma
                ab6 = t6_pool.tile([6, 2], dt.float32, tag="ab6")
                nc.vector.tensor_mul(ab6[:, 0:1], gam_sb[L][:], var[:])
                nc.vector.tensor_mul(musq[:], mu[:], ab6[:, 0:1])
                nc.vector.tensor_sub(ab6[:, 1:2], bet_sb[L][:], musq[:])
                pab = psD.tile([96, 2], dt.float32, tag="pab")
                nc.tensor.matmul(pab[:], selT_sb[:], ab6[:],
                                 start=True, stop=True)
                ab = st_pool.tile([96, 2], dt.float32, tag="ab")
                nc.vector.tensor_copy(ab[:], pab[:])
                # BN apply + relu (in place); fold dis for the next conv
                for u in range(2):
                    nc.scalar.activation(o_t[u][:], o_t[u][:], AF.Relu,
                                         bias=ab[:, 1:2].opt(),
                                         scale=ab[:, 0:1].opt())
                    if L < 2:
                        nc.vector.tensor_mul(o_t[u][:], o_t[u][:], dis_sb[:])
                h_t = [o_t[0], o_t[1]]

            # ================= MLP head =================
            # o -> node-major bf16 (transpose via PE with identity)
            o_bf = mnm_pool.tile([128, NBLK, EW], dt.bfloat16, tag="mnm",
                                 name="o_bf")
            for u in range(2):
                for b0 in range(0, NBLK, 5):
                    nb = min(5, NBLK - b0)
                    pt = ps1.tile([128, 512], dt.float32, tag="ps1",
                                  name="pto")
                    for i in range(nb):
                        c = b0 + i
                        nc.tensor.matmul(pt[:, 96 * i:96 * (i + 1)],
                                         h_t[u][:, 128 * c:128 * (c + 1)],
                                         i96_sb[:], start=True, stop=True)
                    src = pt[:, :96 * nb].rearrange("p (c f) -> p c f", f=96)
                    nc.vector.tensor_copy(
                        o_bf[:, b0:b0 + nb, 96 * u:96 * (u + 1)], src)

            zt = [psD.tile([128, 32], dt.float32, tag="pst", name="zt0"),
                  psD.tile([128, 32], dt.float32, tag="pab", name="zt1")]
            for si in range(STG_BUFS, NSTG):
                stg_tiles[si] = stg_load(si)
            for f in range(H):
                for c in range(NBLK):
                    fc = f * NBLK + c
                    st_t = stg_tiles[fc // 16]
                    rhs = (o_bf[:, c, 0:W]
                           .rearrange("p (g f) -> p f g", f=H)[:, f, :])
                    for h2 in range(2):
                        nc.tensor.matmul(
                            zt[h2][:],
                            st_t[:, fc % 16, 128 * h2:128 * (h2 + 1)],
                            rhs, start=(fc == 0), stop=(fc == H * NBLK - 1))
            zc = [st_pool.tile([128, 32], dt.float32, tag=f"zc{h2}",
                               name=f"zc{h2}") for h2 in range(2)]
            for h2 in range(2):
                nc.vector.tensor_copy(zc[h2][:], zt[h2][:])
                nc.sync.dma_start(mlp_in.ap()[h2], zc[h2][:])
            nc.gpsimd.collective_compute(
                "AllReduce", ALU.add, replica_groups=groups,
                ins=[mlp_in.ap()], outs=[mlp_out.ap()])
            lb1_sb = cpool.tile([128, 2], dt.float32)
            nc.sync.dma_start(lb1_sb[:], lb1h[:])
            lw2_sb = cpool.tile([128, 2, 128], dt.float32)
            nc.sync.dma_start(lw2_sb[:], lw2r[:])
            lw3_sb = cpool.tile([128, 64], dt.float32)
            nc.sync.dma_start(lw3_sb[:], lw3[:])
            lw4_sb = cpool.tile([64, 32], dt.float32)
            nc.sync.dma_start(lw4_sb[:], lw4[:])
            lb2_sb = cpool.tile([128, 1], dt.float32)
            nc.sync.dma_start(lb2_sb[:], lb2c[:])
            lb3_sb = cpool.tile([64, 1], dt.float32)
            nc.sync.dma_start(lb3_sb[:], lb3c[:])
            lb4_sb = cpool.tile([32, 1], dt.float32)
            nc.sync.dma_start(lb4_sb[:], lb4c[:])

            h1 = [st_pool.tile([128, 32], dt.float32, tag=f"h1{h2}",
                               name=f"h1_{h2}") for h2 in range(2)]
            for h2 in range(2):
                nc.sync.dma_start(h1[h2][:], mlp_out.ap()[h2])
                nc.scalar.activation(h1[h2][:], h1[h2][:], AF.Relu,
                                     bias=lb1_sb[:, h2:h2 + 1].opt())
            p2 = psD.tile([128, 32], dt.float32, tag="pst")
            for h2 in range(2):
                nc.tensor.matmul(p2[:], lw2_sb[:, h2, :], h1[h2][:],
                                 start=(h2 == 0), stop=(h2 == 1))
            h2x = st_pool.tile([128, 32], dt.float32, tag="h2x")
            nc.scalar.activation(h2x[:], p2[:], AF.Relu, bias=lb2_sb[:].opt())
            p3 = psD.tile([64, 32], dt.float32, tag="pab")
            nc.tensor.matmul(p3[:], lw3_sb[:], h2x[:], start=True, stop=True)
            h3 = st_pool.tile([64, 32], dt.float32, tag="h3")
            nc.scalar.activation(h3[:], p3[:], AF.Relu, bias=lb3_sb[:].opt())
            p4 = psD.tile([32, 32], dt.float32, tag="pst")
            nc.tensor.matmul(p4[:], lw4_sb[:], h3[:], start=True, stop=True)
            o4 = st_pool.tile([32, 32], dt.float32, tag="o4")
            nc.vector.tensor_scalar_add(o4[:], p4[:], lb4_sb[:].opt())
            nc.sync.dma_start(out_d.ap().rearrange("g c -> c g"), o4[:])

    nc.compile()
    return nc


def _make_in_maps(plan, inputs):
    f32 = np.float32
    bf16 = ml_dtypes.bfloat16
    x = np.asarray(inputs["x"], f32)
    W1 = np.asarray(inputs["W1"], f32)
    W2 = np.asarray(inputs["W2"], f32)
    W3 = np.asarray(inputs["W3"], f32)
    lw1 = np.asarray(inputs["lw1"], f32)
    lw2 = np.asarray(inputs["lw2"], f32)
    lw3 = np.asarray(inputs["lw3"], f32)
    lw4 = np.asarray(inputs["lw4"], f32)

    bw1 = np.kron(np.eye(16, dtype=f32), W1).astype(bf16)   # [48, 96]
    bw2 = np.kron(np.eye(16, dtype=f32), W2).astype(bf16)   # [96, 96]
    bw3 = np.kron(np.eye(16, dtype=f32), W3).astype(bf16)
    i96 = np.eye(96, dtype=f32).astype(bf16)
    sel = np.tile(np.eye(6, dtype=f32), (16, 1))             # [96, 6]
    selT = np.ascontiguousarray(sel.T)                       # [6, 96]
    lw2r = np.ascontiguousarray(
        lw2.reshape(2, 128, 128).transpose(1, 0, 2))         # [128, 2, 128]
    lb1h = np.ascontiguousarray(
        np.asarray(inputs["lb1"], f32).reshape(2, 128).T)    # [128, 2]

    dis = plan["dis"]                                        # [N]
    xg = x.reshape(B, N, DIM)

    common = {
        "bw1": bw1, "bw2": bw2, "bw3": bw3, "i96": i96,
        "sel": sel, "selT": selT,
        "g1": np.asarray(inputs["g1"], f32).reshape(6, 1),
        "be1": np.asarray(inputs["be1"], f32).reshape(6, 1),
        "g2": np.asarray(inputs["g2"], f32).reshape(6, 1),
        "be2": np.asarray(inputs["be2"], f32).reshape(6, 1),
        "g3": np.asarray(inputs["g3"], f32).reshape(6, 1),
        "be3": np.asarray(inputs["be3"], f32).reshape(6, 1),
        "lw2r": lw2r, "lw3": lw3, "lw4": lw4,
        "lb1h": lb1h,
        "lb2c": np.asarray(inputs["lb2"], f32).reshape(128, 1),
        "lb3c": np.asarray(inputs["lb3"], f32).reshape(64, 1),
        "lb4c": np.asarray(inputs["lb4"], f32).reshape(32, 1),
    }

    in_maps = []
    for k in range(NC):
        pc = plan["per_core"][k]
        dk = dis[NS * k:NS * (k + 1)]
        # x feature-major, dis-folded: xq[u][g*3+fi, n]
        xs = xg[:, NS * k:NS * (k + 1), :]              # [32, 6250, 3]
        xq = np.zeros((2, 48, NSP), f32)
        for u in range(2):
            blkv = (xs[16 * u:16 * (u + 1)] * dk[None, :, None])
            xq[u, :, :NS] = blkv.transpose(0, 2, 1).reshape(48, NS)
        disb = np.ones((96, NSP), f32)
        disb[:, :NS] = dk[None, :]
        # lw1 as [partition, (f, chunk), 256] so each staging tile is one
        # contiguous 8KB run per partition
        lw1s = lw1[NS * k * H:NS * (k + 1) * H].reshape(NS, H, 256)
        lw1f = np.zeros((H, NSP, 256), np.float32)
        lw1f[:, :NS, :] = lw1s.transpose(1, 0, 2)
        # [H, NBLK, 128, 256] -> [128, H*NBLK, 256]
        lw1q = (lw1f.reshape(H, NBLK, 128, 256).transpose(2, 0, 1, 3)
                .reshape(128, H * NBLK, 256).astype(bf16))
        m = dict(common)
        m.update({
            "x0": np.ascontiguousarray(xq[0]).astype(bf16),
            "x1": np.ascontiguousarray(xq[1]).astype(bf16),
            "disb": disb.astype(bf16),
            "gmsg0": pc["gmsg0"], "gmsg1": pc["gmsg1"],
            "oh0": pc["oh0"], "oh1": pc["oh1"],
            "lw1q": np.ascontiguousarray(lw1q),
        })
        in_maps.append(m)
    return in_maps


def _get(edge_base):
    key = hash(np.asarray(edge_base).tobytes())
    if key not in _cache:
        plan = _build_plan(np.asarray(edge_base))
        nc = _build_nc(plan)
        _cache[key] = (plan, nc)
    return _cache[key]


def kernel(**inputs):
    from concourse.bass_utils import run_bass_kernel_spmd
    assert int(inputs["num_graphs"]) == B and int(inputs["num_nodes"]) == N
    plan, nc = _get(inputs["edge_base"])
    in_maps = _make_in_maps(plan, inputs)
    trace = os.environ.get("KERNEL_TRACE", "0") == "1"
    res = run_bass_kernel_spmd(nc, in_maps, core_ids=list(range(NC)),
                               trace=trace)
    kernel.last_result = res
    return np.ascontiguousarray(res.results[0]["out"])


# revision 17
# speedup vs baseline: 1.5318x; 1.2147x over previous
"""Trainium2 Bass kernel for nn_Encoder (3-layer GCN + BatchNorm + MLP head).

Sharding: nodes sharded across 8 cores (6250-node slices, all 32 graphs at
full 192-float width).  Per layer:
  - transform runs in bf16 on the PE; the node-major message table is split
    into two half-tables (3200 + 3072 rows/core) that are AllGather'd
    separately so the second AllGather and the first gather/segment-sum pass
    overlap
  - messages are fetched straight from the gathered half-tables with
    dma_gather (512B rows, int16 indices fit because each half-table has
    <32768 rows) over 4 SWDGE queues; no intermediate compaction
  - the segment-sum is a PE matmul against host-built fp8 one-hot matrices;
    pass A (table-0 tokens) evicts partial sums (+self term) to SBUF, pass B
    (table-1 tokens) finishes, scales by dis, and computes BN stats
  - BN statistics are pooled with a 12-float AllReduce
The MLP head contracts the 300000-dim axis with lw1 row-sharded per core
(bf16, streamed through SBUF with prefetch) and a [256,32] AllReduce.
"""
import os
import numpy as np
import ml_dtypes

N = 50000
B = 32
E = 150000
DIM = 3
H = 6
NC = 8
NS = N // NC            # 6250 nodes per core
NSP = 6272              # padded to 49*128
NBLK = NSP // 128       # 49 dst chunks
H0C = 25                # chunks in half-table 0
H1C = NBLK - H0C        # 24
H0 = H0C * 128          # 3200 rows/core in table 0
H1 = H1C * 128          # 3072 rows/core in table 1
EW = 256                # padded bf16 row width (512 bytes)
W = B * H               # 192 payload floats per row
NB = B * N
EPS = 1e-5
CALL = 1024             # gather tokens per call (SWDGE ring limit)
CPC = CALL // 128       # msg chunks per call
NQ = 4                  # SWDGE queues
BLK_PER_W = 4           # dst chunks per psum window
NW = (NBLK + BLK_PER_W - 1) // BLK_PER_W   # 13 windows

_cache = {}


def _wrap_idx(arr):
    """[n] int array -> [128, n/16] int16 device layout (16-wrap, replicated
    for the 8 Q7 cores)."""
    n = len(arr)
    assert n % 16 == 0
    w = arr.reshape(n // 16, 16).T.astype(np.int16)
    return np.ascontiguousarray(np.tile(w, (8, 1)))


def _build_plan(edge_base):
    """Host-side index preprocessing. Returns uniform shapes + per-core data."""
    row = np.asarray(edge_base[0], dtype=np.int64)
    col = np.asarray(edge_base[1], dtype=np.int64)
    deg = (np.bincount(col, minlength=N) + 1).astype(np.float32)
    dis = 1.0 / np.sqrt(deg)

    # per-core edge lists, split by source half-table
    cores = []
    cnt = np.zeros((2, NC, NBLK), np.int64)
    for k in range(NC):
        sel = (col // NS) == k
        src = row[sel]
        dstl = col[sel] - NS * k
        j = src // NS
        off = src - j * NS
        t = (off >= H0).astype(np.int64)
        trow = np.where(t == 0, j * H0 + off, j * H1 + (off - H0))
        b = dstl // 128
        cores.append((t, b, trow, dstl))
        for tt in (0, 1):
            cnt[tt, k] = np.bincount(b[t == tt], minlength=NBLK)

    # uniform (cross-core) chunk counts per (block, table)
    CH = [np.maximum(1, -(-cnt[tt].max(axis=0) // 128)).astype(int)
          for tt in (0, 1)]
    choff = [np.concatenate([[0], np.cumsum(CH[tt])]) for tt in (0, 1)]
    nchunk = [int(CH[tt].sum()) for tt in (0, 1)]
    tok = [nchunk[tt] * 128 for tt in (0, 1)]

    per_core = []
    for k in range(NC):
        t, b, trow, dstl = cores[k]
        gmsg = [np.zeros(tok[tt], np.int64) for tt in (0, 1)]
        oh = [np.zeros((tok[tt], 128), np.float32) for tt in (0, 1)]
        for tt in (0, 1):
            m = t == tt
            bb, rr, dd = b[m], trow[m], dstl[m]
            order = np.argsort(bb, kind="stable")
            bb, rr, dd = bb[order], rr[order], dd[order]
            starts = np.searchsorted(bb, np.arange(NBLK))
            ends = np.searchsorted(bb, np.arange(NBLK) + 1)
            for jb in range(NBLK):
                s, e = int(starts[jb]), int(ends[jb])
                n = e - s
                base = int(choff[tt][jb]) * 128
                assert n <= CH[tt][jb] * 128
                gmsg[tt][base:base + n] = rr[s:e]
                oh[tt][base + np.arange(n), dd[s:e] - 128 * jb] = 1.0
        pc = {}
        for tt in (0, 1):
            oh_dev = (oh[tt].reshape(nchunk[tt], 128, 128)
                      .transpose(1, 0, 2))
            pc[f"gmsg{tt}"] = _wrap_idx(gmsg[tt])
            pc[f"oh{tt}"] = np.ascontiguousarray(
                oh_dev.astype(ml_dtypes.float8_e4m3))
        per_core.append(pc)

    return {
        "dis": dis, "CH": CH, "choff": choff, "nchunk": nchunk, "tok": tok,
        "per_core": per_core,
    }


def _build_nc(plan):
    import concourse.bacc as bacc
    import concourse.mybir as mybir
    import concourse.tile as tile

    dt = mybir.dt
    AF = mybir.ActivationFunctionType
    ALU = mybir.AluOpType
    CH = plan["CH"]
    CHOFF = plan["choff"]
    NCHUNK = plan["nchunk"]
    TOK = plan["tok"]
    NCALL = [-(-TOK[tt] // CALL) for tt in (0, 1)]
    NOHT = [-(-NCHUNK[tt] // 16) for tt in (0, 1)]

    nc = bacc.Bacc("TRN2", target_bir_lowering=False, debug=False,
                   num_devices=NC, enable_asserts=False, num_swdge_queues=NQ)

    # ---------------- I/O ----------------
    def inp(name, shape, d):
        return nc.dram_tensor(name, shape, d, kind="ExternalInput")

    x0 = inp("x0", [48, NSP], dt.bfloat16)
    x1 = inp("x1", [48, NSP], dt.bfloat16)
    disb = inp("disb", [96, NSP], dt.bfloat16)
    gmsg = [inp(f"gmsg{tt}", [128, TOK[tt] // 16], dt.int16) for tt in (0, 1)]
    oh_in = [inp(f"oh{tt}", [128, NCHUNK[tt], 128], dt.float8e4)
             for tt in (0, 1)]
    bw = [inp("bw1", [48, 96], dt.bfloat16),
          inp("bw2", [96, 96], dt.bfloat16),
          inp("bw3", [96, 96], dt.bfloat16)]
    i96 = inp("i96", [96, 96], dt.bfloat16)
    sel = inp("sel", [96, 6], dt.float32)
    selT = inp("selT", [6, 96], dt.float32)
    gam = [inp(f"g{i}", [6, 1], dt.float32) for i in (1, 2, 3)]
    bet = [inp(f"be{i}", [6, 1], dt.float32) for i in (1, 2, 3)]
    lw1q = inp("lw1q", [128, H * NBLK, 256], dt.bfloat16)
    lw2r = inp("lw2r", [128, 2, 128], dt.float32)
    lw3 = inp("lw3", [128, 64], dt.float32)
    lw4 = inp("lw4", [64, 32], dt.float32)
    lb1h = inp("lb1h", [128, 2], dt.float32)
    lb2c = inp("lb2c", [128, 1], dt.float32)
    lb3c = inp("lb3c", [64, 1], dt.float32)
    lb4c = inp("lb4c", [32, 1], dt.float32)
    out_d = nc.dram_tensor("out", [B, 32], dt.float32, kind="ExternalOutput")

    m_hbm = [nc.dram_tensor("m_hbm0", [H0, EW], dt.bfloat16, kind="Internal"),
             nc.dram_tensor("m_hbm1", [H1, EW], dt.bfloat16, kind="Internal")]
    # one Shared tensor holding both gathered half-tables; the gathers read
    # slices of it so they serialize behind BOTH AllGathers (overlapping the
    # gathers with the collectives measured as a net loss: the mixed
    # scattered-read + CC traffic runs below the sum of the solo rates)
    m_full_all = nc.dram_tensor("m_full_all", [NC * NSP, EW], dt.bfloat16,
                                kind="Internal", addr_space="Shared")
    m_full = [m_full_all.ap()[0:NC * H0, :],
              m_full_all.ap()[NC * H0:NC * NSP, :]]
    st_in = [nc.dram_tensor(f"st_in{i}", [6, 2], dt.float32, kind="Internal")
             for i in range(3)]
    st_out = [nc.dram_tensor(f"st_out{i}", [6, 2], dt.float32, kind="Internal",
                             addr_space="Shared") for i in range(3)]
    mlp_in = nc.dram_tensor("mlp_in", [2, 128, 32], dt.float32, kind="Internal")
    mlp_out = nc.dram_tensor("mlp_out", [2, 128, 32], dt.float32,
                             kind="Internal", addr_space="Shared")

    groups = [list(range(NC))]
    NSTG = (H * NBLK + 15) // 16       # lw1 staging tiles
    STG_BUFS = 5

    with tile.TileContext(nc) as tc:
        with (
            tc.tile_pool(name="const", bufs=1) as cpool,
            tc.tile_pool(name="ho", bufs=1) as ho_pool,
            tc.tile_pool(name="mfm", bufs=1) as mfm_pool,
            tc.tile_pool(name="mnm", bufs=1) as mnm_pool,
            tc.tile_pool(name="msg", bufs=8) as msg_pool,
            tc.tile_pool(name="ohp", bufs=6) as oh_pool,
            tc.tile_pool(name="ysc", bufs=2) as y_pool,
            tc.tile_pool(name="st", bufs=1) as st_pool,
            tc.tile_pool(name="t6", bufs=1) as t6_pool,
            tc.tile_pool(name="stg", bufs=STG_BUFS) as stg_pool,
            tc.tile_pool(name="psA", bufs=4, space="PSUM") as psA,
            tc.tile_pool(name="ps1", bufs=2, space="PSUM") as ps1,
            tc.tile_pool(name="psD", bufs=1, space="PSUM") as psD,
        ):
            # ---------- setup ----------
            gmsg_sb = []
            for tt in (0, 1):
                t = cpool.tile([128, TOK[tt] // 16], dt.int16,
                               tag=f"gmsg{tt}", name=f"gmsg_sb{tt}")
                nc.sync.dma_start(t[:], gmsg[tt][:])
                gmsg_sb.append(t)
            bw_sb = []
            for i in range(3):
                t = cpool.tile([48 if i == 0 else 96, 96], dt.bfloat16,
                               tag=f"bw{i}", name=f"bw_sb{i}")
                nc.sync.dma_start(t[:], bw[i][:])
                bw_sb.append(t)
            i96_sb = cpool.tile([96, 96], dt.bfloat16)
            nc.sync.dma_start(i96_sb[:], i96[:])
            sel_sb = cpool.tile([96, 6], dt.float32)
            nc.sync.dma_start(sel_sb[:], sel[:])
            selT_sb = cpool.tile([6, 96], dt.float32)
            nc.sync.dma_start(selT_sb[:], selT[:])
            gam_sb, bet_sb = [], []
            for i in range(3):
                g_t = cpool.tile([6, 1], dt.float32, tag=f"gam{i}",
                                 name=f"gam_sb{i}")
                nc.sync.dma_start(g_t[:], gam[i][:])
                gam_sb.append(g_t)
                b_t = cpool.tile([6, 1], dt.float32, tag=f"bet{i}",
                                 name=f"bet_sb{i}")
                nc.sync.dma_start(b_t[:], bet[i][:])
                bet_sb.append(b_t)
            eps_sb = cpool.tile([6, 1], dt.float32, name="eps_sb")
            nc.vector.memset(eps_sb[:], EPS)
            dis_sb = cpool.tile([96, NSP], dt.bfloat16)
            nc.sync.dma_start(dis_sb[:], disb[:])

            # x feature-major, already dis-scaled on host
            h_t = [ho_pool.tile([48, NSP], dt.bfloat16, tag=f"ho{u}",
                                name=f"x_sb{u}") for u in range(2)]
            nc.sync.dma_start(h_t[0][:], x0[:])
            nc.sync.dma_start(h_t[1][:], x1[:])

            def stg_load(si):
                nch = min(16, H * NBLK - si * 16)
                t = stg_pool.tile([128, nch, 256], dt.bfloat16, tag="stg",
                                  name=f"stg{si}")
                nc.scalar.dma_start(t[:], lw1q[:, si * 16:si * 16 + nch, :])
                return t

            # ================= conv layers =================
            for L in range(3):
                kin = 48 if L == 0 else 96
                m_nm = mnm_pool.tile([128, NBLK, EW], dt.bfloat16, tag="mnm",
                                     name=f"mnm_L{L}")
                if L == 0:
                    nc.vector.memset(m_nm[:, :, W:EW], 0.0)

                def emit_mnm(c0, c1, on_group, L=L, m_nm=m_nm, h_t=h_t):
                    for b0 in range(c0, c1, 5):
                        nb = min(5, c1 - b0)
                        for u in range(2):
                            pt = ps1.tile([128, 512], dt.float32, tag="ps1",
                                          name=f"ptb_L{L}")
                            for i in range(nb):
                                c = b0 + i
                                nc.tensor.matmul(
                                    pt[:, 96 * i:96 * (i + 1)],
                                    h_t[u][:, 128 * c:128 * (c + 1)],
                                    bw_sb[L][:], start=True, stop=True)
                            src = (pt[:, :96 * nb]
                                   .rearrange("p (c f) -> p c f", f=96))
                            nc.vector.tensor_copy(
                                m_nm[:, b0:b0 + nb, 96 * u:96 * (u + 1)], src)
                        on_group(b0, nb)

                # half A -> AllGather0, half B -> AllGather1; the m_hbm
                # writes go out per 5-chunk group, overlapping the transform
                def m_write(b0, nb, L=L, m_nm=m_nm):
                    half = 0 if b0 < H0C else 1
                    base = b0 - (0 if half == 0 else H0C)
                    nc.sync.dma_start(
                        m_hbm[half].ap()[128 * base:128 * (base + nb), :]
                        .rearrange("(c p) e -> p c e", p=128),
                        m_nm[:, b0:b0 + nb, :])

                emit_mnm(0, H0C, on_group=m_write)
                nc.gpsimd.collective_compute(
                    "AllGather", ALU.bypass, replica_groups=groups,
                    ins=[m_hbm[0].ap()], outs=[m_full[0]])
                emit_mnm(H0C, NBLK, on_group=m_write)
                nc.gpsimd.collective_compute(
                    "AllGather", ALU.bypass, replica_groups=groups,
                    ins=[m_hbm[1].ap()], outs=[m_full[1]])

                # m_fm (feature-major self term) runs under the AllGathers
                m_fm = [mfm_pool.tile([96, NSP], dt.bfloat16, tag=f"mfm{u}",
                                      name=f"mfm_L{L}_{u}") for u in range(2)]
                for u in range(2):
                    for c0 in range(0, NSP, 512):
                        cw = min(512, NSP - c0)
                        pt = ps1.tile([128, 512], dt.float32, tag="ps1",
                                      name=f"ptf_L{L}")
                        nc.tensor.matmul(pt[0:96, :cw], bw_sb[L][:],
                                         h_t[u][:, c0:c0 + cw],
                                         start=True, stop=True)
                        nc.vector.tensor_copy(m_fm[u][:, c0:c0 + cw],
                                              pt[0:96, :cw])

                # prefetch lw1 staging during L3's gather passes
                stg_tiles = {}
                if L == 2:
                    for si in range(STG_BUFS):
                        stg_tiles[si] = stg_load(si)

                # gather calls + one-hot loads, emitted in consumption order
                # (sorted by the first window each one feeds)
                def first_window(tt, q0):
                    jb = int(np.searchsorted(CHOFF[tt], q0, side="right")) - 1
                    return jb // BLK_PER_W

                calls = sorted(
                    [(first_window(tt, ci * CPC), tt, ci)
                     for tt in (0, 1) for ci in range(NCALL[tt])])
                msg_tiles = [[None] * NCALL[tt] for tt in (0, 1)]
                for qi, (_, tt, ci) in enumerate(calls):
                    nch = min(CPC, NCHUNK[tt] - ci * CPC)
                    t = msg_pool.tile([128, nch, EW], dt.bfloat16,
                                      tag="msg", name=f"msg_L{L}_{tt}_{ci}")
                    nc.gpsimd.dma_gather(
                        t[:], m_full[tt],
                        gmsg_sb[tt][:, ci * (CALL // 16):
                                    ci * (CALL // 16) + nch * 8],
                        num_idxs=nch * 128, num_idxs_reg=nch * 128,
                        elem_size=EW, queue_num=qi % NQ)
                    msg_tiles[tt][ci] = t
                ohs = sorted(
                    [(first_window(tt, ti * 16), tt, ti)
                     for tt in (0, 1) for ti in range(NOHT[tt])])
                oh_tiles = [[None] * NOHT[tt] for tt in (0, 1)]
                for _, tt, ti in ohs:
                    nch = min(16, NCHUNK[tt] - ti * 16)
                    t = oh_pool.tile([128, nch, 128], dt.float8e4,
                                     tag="oh", name=f"oh_L{L}_{tt}_{ti}")
                    nc.sync.dma_start(
                        t[:], oh_in[tt][:, ti * 16:ti * 16 + nch, :])
                    oh_tiles[tt][ti] = t

                o_t = [ho_pool.tile([96, NSP], dt.bfloat16, tag=f"ho{u}",
                                    name=f"o_L{L}_{u}") for u in range(2)]
                S_t = st_pool.tile([96, 4 * NW], dt.float32, tag="S")

                def seg_window(w, L=L, msg_tiles=msg_tiles,
                               oh_tiles=oh_tiles, o_t=o_t,
                               m_fm=m_fm, S_t=S_t):
                    jlo = w * BLK_PER_W
                    jhi = min(jlo + BLK_PER_W, NBLK)
                    pw = [psA.tile([96, 512], dt.float32, tag="psA",
                                   name=f"pw_L{L}_{w}_{uu}")
                          for uu in range(2)]
                    for j in range(jlo, jhi):
                        qs = [(tt, int(CHOFF[tt][j]) + c)
                              for tt in (0, 1) for c in range(int(CH[tt][j]))]
                        for k, (tt, q) in enumerate(qs):
                            mt = msg_tiles[tt][q // CPC]
                            ot = oh_tiles[tt][q // 16]
                            for u in range(2):
                                nc.tensor.matmul(
                                    pw[u][:, 128 * (j - jlo):
                                          128 * (j - jlo + 1)],
                                    mt[:, q % CPC, 96 * u:96 * (u + 1)],
                                    ot[:, q % 16, :],
                                    start=(k == 0),
                                    stop=(k == len(qs) - 1))
                    c0 = 512 * w
                    cw = min(512, NS - c0)    # stats over real nodes only
                    cwf = min(512, NSP - c0)
                    for u in range(2):
                        y = y_pool.tile([96, 512], dt.float32, tag="y")
                        nc.vector.tensor_add(
                            y[:, :cwf], pw[u][:, :cwf], m_fm[u][:, c0:c0 + cwf])
                        nc.vector.tensor_mul(
                            o_t[u][:, c0:c0 + cwf], y[:, :cwf],
                            dis_sb[:, c0:c0 + cwf])
                        # stats: sum + sum-of-squares via Act accumulators
                        y2 = y_pool.tile([96, 512], dt.bfloat16, tag="y2")
                        nc.scalar.activation(
                            y2[:, :cw], o_t[u][:, c0:c0 + cw], AF.Copy,
                            accum_out=S_t[:, 2 * w + u:2 * w + u + 1])
                        nc.scalar.activation(
                            y2[:, :cw], o_t[u][:, c0:c0 + cw], AF.Square,
                            accum_out=S_t[:, 2 * (NW + w) + u:
                                          2 * (NW + w) + u + 1])

                for w in range(NW):
                    seg_window(w)

                # ---------- BN stats + apply ----------
                st2 = st_pool.tile([96, 4], dt.float32, tag="st2")
                for u in range(2):
                    nc.vector.tensor_reduce(
                        st2[:, u:u + 1],
                        S_t[:, :2 * NW].rearrange("p (w u) -> p u w", u=2)
                        [:, u, :], axis=mybir.AxisListType.X, op=ALU.add)
                    nc.vector.tensor_reduce(
                        st2[:, 2 + u:3 + u],
                        S_t[:, 2 * NW:4 * NW]
                        .rearrange("p (w u) -> p u w", u=2)[:, u, :],
                        axis=mybir.AxisListType.X, op=ALU.add)
                pst = psD.tile([6, 2], dt.float32, tag="pst")
                for u in range(2):
                    nc.tensor.matmul(
                        pst[:], sel_sb[:],
                        st2[:, :].rearrange("p (a u) -> p u a", u=2)[:, u, :],
                        start=(u == 0), stop=(u == 1))
                stt = t6_pool.tile([6, 2], dt.float32, tag="stt")
                nc.vector.tensor_copy(stt[:], pst[:])
                nc.sync.dma_start(st_in[L].ap(), stt[:])
                nc.gpsimd.collective_compute(
                    "AllReduce", ALU.add, replica_groups=groups,
                    ins=[st_in[L].ap()], outs=[st_out[L].ap()])
                sto = t6_pool.tile([6, 2], dt.float32, tag="sto")
                nc.sync.dma_start(sto[:], st_out[L].ap())
                mu = t6_pool.tile([6, 1], dt.float32, tag="mu")
                nc.vector.tensor_scalar_mul(mu[:], sto[:, 0:1], 1.0 / NB)
                var = t6_pool.tile([6, 1], dt.float32, tag="var")
                nc.vector.tensor_scalar_mul(var[:], sto[:, 1:2], 1.0 / NB)
                musq = t6_pool.tile([6, 1], dt.float32, tag="musq")
                nc.vector.tensor_mul(musq[:], mu[:], mu[:])
                nc.vector.tensor_sub(var[:], var[:], musq[:])
                nc.scalar.activation(var[:], var[:], AF.Sqrt,
                                     bias=eps_sb[:].opt())
                nc.vector.reciprocal(var[:], var[:])     # var := 1/sigma
                ab6 = t6_pool.tile([6, 2], dt.float32, tag="ab6")
                nc.vector.tensor_mul(ab6[:, 0:1], gam_sb[L][:], var[:])
                nc.vector.tensor_mul(musq[:], mu[:], ab6[:, 0:1])
                nc.vector.tensor_sub(ab6[:, 1:2], bet_sb[L][:], musq[:])
                pab = psD.tile([96, 2], dt.float32, tag="pab")
                nc.tensor.matmul(pab[:], selT_sb[:], ab6[:],
                                 start=True, stop=True)
                ab = st_pool.tile([96, 2], dt.float32, tag="ab")
                nc.vector.tensor_copy(ab[:], pab[:])
                # BN apply + relu (in place); fold dis for the next conv
                for u in range(2):
                    nc.scalar.activation(o_t[u][:], o_t[u][:], AF.Relu,
                                         bias=ab[:, 1:2].opt(),
                                         scale=ab[:, 0:1].opt())
                    if L < 2:
                        nc.vector.tensor_mul(o_t[u][:], o_t[u][:], dis_sb[:])
                h_t = [o_t[0], o_t[1]]

            # ================= MLP head =================
            # o -> node-major bf16 (transpose via PE with identity)
            o_bf = mnm_pool.tile([128, NBLK, EW], dt.bfloat16, tag="mnm",
                                 name="o_bf")
            for u in range(2):
                for b0 in range(0, NBLK, 5):
                    nb = min(5, NBLK - b0)
                    pt = ps1.tile([128, 512], dt.float32, tag="ps1",
                                  name="pto")
                    for i in range(nb):
                        c = b0 + i
                        nc.tensor.matmul(pt[:, 96 * i:96 * (i + 1)],
                                         h_t[u][:, 128 * c:128 * (c + 1)],
                                         i96_sb[:], start=True, stop=True)
                    src = pt[:, :96 * nb].rearrange("p (c f) -> p c f", f=96)
                    nc.vector.tensor_copy(
                        o_bf[:, b0:b0 + nb, 96 * u:96 * (u + 1)], src)

            zt = [psD.tile([128, 32], dt.float32, tag="pst", name="zt0"),
                  psD.tile([128, 32], dt.float32, tag="pab", name="zt1")]
            for si in range(STG_BUFS, NSTG):
                stg_tiles[si] = stg_load(si)
            for f in range(H):
                for c in range(NBLK):
                    fc = f * NBLK + c
                    st_t = stg_tiles[fc // 16]
                    rhs = (o_bf[:, c, 0:W]
                           .rearrange("p (g f) -> p f g", f=H)[:, f, :])
                    for h2 in range(2):
                        nc.tensor.matmul(
                            zt[h2][:],
                            st_t[:, fc % 16, 128 * h2:128 * (h2 + 1)],
                            rhs, start=(fc == 0), stop=(fc == H * NBLK - 1))
            zc = [st_pool.tile([128, 32], dt.float32, tag=f"zc{h2}",
                               name=f"zc{h2}") for h2 in range(2)]
            for h2 in range(2):
                nc.vector.tensor_copy(zc[h2][:], zt[h2][:])
                nc.sync.dma_start(mlp_in.ap()[h2], zc[h2][:])
            nc.gpsimd.collective_compute(
                "AllReduce", ALU.add, replica_groups=groups,
                ins=[mlp_in.ap()], outs=[mlp_out.ap()])
            lb1_sb = cpool.tile([128, 2], dt.float32)
            nc.sync.dma_start(lb1_sb[:], lb1h[:])
            lw2_sb = cpool.tile([128, 2, 128], dt.float32)
            nc.sync.dma_start(lw2_sb[:], lw2r[:])
            lw3_sb = cpool.tile([128, 64], dt.float32)
            nc.sync.dma_start(lw3_sb[:], lw3[:])
            lw4_sb = cpool.tile([64, 32], dt.float32)
            nc.sync.dma_start(lw4_sb[:], lw4[:])
            lb2_sb = cpool.tile([128, 1], dt.float32)
            nc.sync.dma_start(lb2_sb[:], lb2c[:])
            lb3_sb = cpool.tile([64, 1], dt.float32)
            nc.sync.dma_start(lb3_sb[:], lb3c[:])
            lb4_sb = cpool.tile([32, 1], dt.float32)
            nc.sync.dma_start(lb4_sb[:], lb4c[:])

            h1 = [st_pool.tile([128, 32], dt.float32, tag=f"h1{h2}",
                               name=f"h1_{h2}") for h2 in range(2)]
            for h2 in range(2):
                nc.sync.dma_start(h1[h2][:], mlp_out.ap()[h2])
                nc.scalar.activation(h1[h2][:], h1[h2][:], AF.Relu,
                                     bias=lb1_sb[:, h2:h2 + 1].opt())
            p2 = psD.tile([128, 32], dt.float32, tag="pst")
            for h2 in range(2):
                nc.tensor.matmul(p2[:], lw2_sb[:, h2, :], h1[h2][:],
                                 start=(h2 == 0), stop=(h2 == 1))
            h2x = st_pool.tile([128, 32], dt.float32, tag="h2x")
            nc.scalar.activation(h2x[:], p2[:], AF.Relu, bias=lb2_sb[:].opt())
            p3 = psD.tile([64, 32], dt.float32, tag="pab")
            nc.tensor.matmul(p3[:], lw3_sb[:], h2x[:], start=True, stop=True)
            h3 = st_pool.tile([64, 32], dt.float32, tag="h3")
            nc.scalar.activation(h3[:], p3[:], AF.Relu, bias=lb3_sb[:].opt())
            p4 = psD.tile([32, 32], dt.float32, tag="pst")
            nc.tensor.matmul(p4[:], lw4_sb[:], h3[:], start=True, stop=True)
            o4 = st_pool.tile([32, 32], dt.float32, tag="o4")
            nc.vector.tensor_scalar_add(o4[:], p4[:], lb4_sb[:].opt())
            nc.sync.dma_start(out_d.ap().rearrange("g c -> c g"), o4[:])

    nc.compile()
    return nc


def _make_in_maps(plan, inputs):
    f32 = np.float32
    bf16 = ml_dtypes.bfloat16
    x = np.asarray(inputs["x"], f32)
    W1 = np.asarray(inputs["W1"], f32)
    W2 = np.asarray(inputs["W2"], f32)
    W3 = np.asarray(inputs["W3"], f32)
    lw1 = np.asarray(inputs["lw1"], f32)
    lw2 = np.asarray(inputs["lw2"], f32)
    lw3 = np.asarray(inputs["lw3"], f32)
    lw4 = np.asarray(inputs["lw4"], f32)

    bw1 = np.kron(np.eye(16, dtype=f32), W1).astype(bf16)   # [48, 96]
    bw2 = np.kron(np.eye(16, dtype=f32), W2).astype(bf16)   # [96, 96]
    bw3 = np.kron(np.eye(16, dtype=f32), W3).astype(bf16)
    i96 = np.eye(96, dtype=f32).astype(bf16)
    sel = np.tile(np.eye(6, dtype=f32), (16, 1))             # [96, 6]
    selT = np.ascontiguousarray(sel.T)                       # [6, 96]
    lw2r = np.ascontiguousarray(
        lw2.reshape(2, 128, 128).transpose(1, 0, 2))         # [128, 2, 128]
    lb1h = np.ascontiguousarray(
        np.asarray(inputs["lb1"], f32).reshape(2, 128).T)    # [128, 2]

    dis = plan["dis"]                                        # [N]
    xg = x.reshape(B, N, DIM)

    common = {
        "bw1": bw1, "bw2": bw2, "bw3": bw3, "i96": i96,
        "sel": sel, "selT": selT,
        "g1": np.asarray(inputs["g1"], f32).reshape(6, 1),
        "be1": np.asarray(inputs["be1"], f32).reshape(6, 1),
        "g2": np.asarray(inputs["g2"], f32).reshape(6, 1),
        "be2": np.asarray(inputs["be2"], f32).reshape(6, 1),
        "g3": np.asarray(inputs["g3"], f32).reshape(6, 1),
        "be3": np.asarray(inputs["be3"], f32).reshape(6, 1),
        "lw2r": lw2r, "lw3": lw3, "lw4": lw4,
        "lb1h": lb1h,
        "lb2c": np.asarray(inputs["lb2"], f32).reshape(128, 1),
        "lb3c": np.asarray(inputs["lb3"], f32).reshape(64, 1),
        "lb4c": np.asarray(inputs["lb4"], f32).reshape(32, 1),
    }

    in_maps = []
    for k in range(NC):
        pc = plan["per_core"][k]
        dk = dis[NS * k:NS * (k + 1)]
        # x feature-major, dis-folded: xq[u][g*3+fi, n]
        xs = xg[:, NS * k:NS * (k + 1), :]              # [32, 6250, 3]
        xq = np.zeros((2, 48, NSP), f32)
        for u in range(2):
            blkv = (xs[16 * u:16 * (u + 1)] * dk[None, :, None])
            xq[u, :, :NS] = blkv.transpose(0, 2, 1).reshape(48, NS)
        disb = np.ones((96, NSP), f32)
        disb[:, :NS] = dk[None, :]
        # lw1 as [partition, (f, chunk), 256] so each staging tile is one
        # contiguous 8KB run per partition
        lw1s = lw1[NS * k * H:NS * (k + 1) * H].reshape(NS, H, 256)
        lw1f = np.zeros((H, NSP, 256), np.float32)
        lw1f[:, :NS, :] = lw1s.transpose(1, 0, 2)
        # [H, NBLK, 128, 256] -> [128, H*NBLK, 256]
        lw1q = (lw1f.reshape(H, NBLK, 128, 256).transpose(2, 0, 1, 3)
                .reshape(128, H * NBLK, 256).astype(bf16))
        m = dict(common)
        m.update({
            "x0": np.ascontiguousarray(xq[0]).astype(bf16),
            "x1": np.ascontiguousarray(xq[1]).astype(bf16),
            "disb": disb.astype(bf16),
            "gmsg0": pc["gmsg0"], "gmsg1": pc["gmsg1"],
            "oh0": pc["oh0"], "oh1": pc["oh1"],
            "lw1q": np.ascontiguousarray(lw1q),
        })
        in_maps.append(m)
    return in_maps


def _get(edge_base):
    key = hash(np.asarray(edge_base).tobytes())
    if key not in _cache:
        plan = _build_plan(np.asarray(edge_base))
        nc = _build_nc(plan)
        _cache[key] = (plan, nc)
    return _cache[key]


def kernel(**inputs):
    from concourse.bass_utils import run_bass_kernel_spmd
    assert int(inputs["num_graphs"]) == B and int(inputs["num_nodes"]) == N
    plan, nc = _get(inputs["edge_base"])
    in_maps = _make_in_maps(plan, inputs)
    trace = os.environ.get("KERNEL_TRACE", "0") == "1"
    res = run_bass_kernel_spmd(nc, in_maps, core_ids=list(range(NC)),
                               trace=trace)
    kernel.last_result = res
    return np.ascontiguousarray(res.results[0]["out"])


# revision 19
# speedup vs baseline: 1.5472x; 1.0100x over previous
"""Trainium2 Bass kernel for nn_Encoder (3-layer GCN + BatchNorm + MLP head).

Sharding: nodes sharded across 8 cores (6250-node slices, all 32 graphs at
full 192-float width).  Per layer:
  - transform runs in bf16 on the PE; the node-major message table is split
    into two half-tables (3200 + 3072 rows/core) that are AllGather'd
    separately so the second AllGather and the first gather/segment-sum pass
    overlap
  - messages are fetched straight from the gathered half-tables with
    dma_gather (512B rows, int16 indices fit because each half-table has
    <32768 rows) over 4 SWDGE queues; no intermediate compaction
  - the segment-sum is a PE matmul against host-built fp8 one-hot matrices;
    pass A (table-0 tokens) evicts partial sums (+self term) to SBUF, pass B
    (table-1 tokens) finishes, scales by dis, and computes BN stats
  - BN statistics are pooled with a 12-float AllReduce
The MLP head contracts the 300000-dim axis with lw1 row-sharded per core
(bf16, streamed through SBUF with prefetch) and a [256,32] AllReduce.
"""
import os
import numpy as np
import ml_dtypes

N = 50000
B = 32
E = 150000
DIM = 3
H = 6
NC = 8
NS = N // NC            # 6250 nodes per core
NSP = 6272              # padded to 49*128
NBLK = NSP // 128       # 49 dst chunks
H0C = 25                # chunks in half-table 0
H1C = NBLK - H0C        # 24
H0 = H0C * 128          # 3200 rows/core in table 0
H1 = H1C * 128          # 3072 rows/core in table 1
EW = 256                # padded bf16 row width (512 bytes)
W = B * H               # 192 payload floats per row
NB = B * N
EPS = 1e-5
CALL = 1024             # gather tokens per call (SWDGE ring limit)
CPC = CALL // 128       # msg chunks per call
NQ = 4                  # SWDGE queues
BLK_PER_W = 4           # dst chunks per psum window
NW = (NBLK + BLK_PER_W - 1) // BLK_PER_W   # 13 windows

_cache = {}


def _wrap_idx(arr):
    """[n] int array -> [128, n/16] int16 device layout (16-wrap, replicated
    for the 8 Q7 cores)."""
    n = len(arr)
    assert n % 16 == 0
    w = arr.reshape(n // 16, 16).T.astype(np.int16)
    return np.ascontiguousarray(np.tile(w, (8, 1)))


def _build_plan(edge_base):
    """Host-side index preprocessing. Returns uniform shapes + per-core data."""
    row = np.asarray(edge_base[0], dtype=np.int64)
    col = np.asarray(edge_base[1], dtype=np.int64)
    deg = (np.bincount(col, minlength=N) + 1).astype(np.float32)
    dis = 1.0 / np.sqrt(deg)

    # per-core edge lists, split by source half-table
    cores = []
    cnt = np.zeros((2, NC, NBLK), np.int64)
    for k in range(NC):
        sel = (col // NS) == k
        src = row[sel]
        dstl = col[sel] - NS * k
        j = src // NS
        off = src - j * NS
        t = (off >= H0).astype(np.int64)
        trow = np.where(t == 0, j * H0 + off, j * H1 + (off - H0))
        b = dstl // 128
        cores.append((t, b, trow, dstl))
        for tt in (0, 1):
            cnt[tt, k] = np.bincount(b[t == tt], minlength=NBLK)

    # uniform (cross-core) chunk counts per (block, table)
    CH = [np.maximum(1, -(-cnt[tt].max(axis=0) // 128)).astype(int)
          for tt in (0, 1)]
    choff = [np.concatenate([[0], np.cumsum(CH[tt])]) for tt in (0, 1)]
    nchunk = [int(CH[tt].sum()) for tt in (0, 1)]
    tok = [nchunk[tt] * 128 for tt in (0, 1)]

    per_core = []
    for k in range(NC):
        t, b, trow, dstl = cores[k]
        gmsg = [np.zeros(tok[tt], np.int64) for tt in (0, 1)]
        oh = [np.zeros((tok[tt], 128), np.float32) for tt in (0, 1)]
        for tt in (0, 1):
            m = t == tt
            bb, rr, dd = b[m], trow[m], dstl[m]
            order = np.lexsort((rr, bb))
            bb, rr, dd = bb[order], rr[order], dd[order]
            starts = np.searchsorted(bb, np.arange(NBLK))
            ends = np.searchsorted(bb, np.arange(NBLK) + 1)
            for jb in range(NBLK):
                s, e = int(starts[jb]), int(ends[jb])
                n = e - s
                base = int(choff[tt][jb]) * 128
                assert n <= CH[tt][jb] * 128
                gmsg[tt][base:base + n] = rr[s:e]
                oh[tt][base + np.arange(n), dd[s:e] - 128 * jb] = 1.0
        pc = {}
        for tt in (0, 1):
            oh_dev = (oh[tt].reshape(nchunk[tt], 128, 128)
                      .transpose(1, 0, 2))
            pc[f"gmsg{tt}"] = _wrap_idx(gmsg[tt])
            pc[f"oh{tt}"] = np.ascontiguousarray(
                oh_dev.astype(ml_dtypes.float8_e4m3))
        per_core.append(pc)

    return {
        "dis": dis, "CH": CH, "choff": choff, "nchunk": nchunk, "tok": tok,
        "per_core": per_core,
    }


def _build_nc(plan):
    import concourse.bacc as bacc
    import concourse.mybir as mybir
    import concourse.tile as tile

    dt = mybir.dt
    AF = mybir.ActivationFunctionType
    ALU = mybir.AluOpType
    CH = plan["CH"]
    CHOFF = plan["choff"]
    NCHUNK = plan["nchunk"]
    TOK = plan["tok"]
    NCALL = [-(-TOK[tt] // CALL) for tt in (0, 1)]
    NOHT = [-(-NCHUNK[tt] // 16) for tt in (0, 1)]

    nc = bacc.Bacc("TRN2", target_bir_lowering=False, debug=False,
                   num_devices=NC, enable_asserts=False, num_swdge_queues=NQ)

    # ---------------- I/O ----------------
    def inp(name, shape, d):
        return nc.dram_tensor(name, shape, d, kind="ExternalInput")

    x0 = inp("x0", [48, NSP], dt.bfloat16)
    x1 = inp("x1", [48, NSP], dt.bfloat16)
    disb = inp("disb", [96, NSP], dt.bfloat16)
    gmsg = [inp(f"gmsg{tt}", [128, TOK[tt] // 16], dt.int16) for tt in (0, 1)]
    oh_in = [inp(f"oh{tt}", [128, NCHUNK[tt], 128], dt.float8e4)
             for tt in (0, 1)]
    bw = [inp("bw1", [48, 96], dt.bfloat16),
          inp("bw2", [96, 96], dt.bfloat16),
          inp("bw3", [96, 96], dt.bfloat16)]
    i96 = inp("i96", [96, 96], dt.bfloat16)
    sel = inp("sel", [96, 6], dt.float32)
    selT = inp("selT", [6, 96], dt.float32)
    gam = [inp(f"g{i}", [6, 1], dt.float32) for i in (1, 2, 3)]
    bet = [inp(f"be{i}", [6, 1], dt.float32) for i in (1, 2, 3)]
    lw1q = inp("lw1q", [128, H * NBLK, 256], dt.bfloat16)
    lw2r = inp("lw2r", [128, 2, 128], dt.float32)
    lw3 = inp("lw3", [128, 64], dt.float32)
    lw4 = inp("lw4", [64, 32], dt.float32)
    lb1h = inp("lb1h", [128, 2], dt.float32)
    lb2c = inp("lb2c", [128, 1], dt.float32)
    lb3c = inp("lb3c", [64, 1], dt.float32)
    lb4c = inp("lb4c", [32, 1], dt.float32)
    out_d = nc.dram_tensor("out", [B, 32], dt.float32, kind="ExternalOutput")

    m_hbm = [nc.dram_tensor("m_hbm0", [H0, EW], dt.bfloat16, kind="Internal"),
             nc.dram_tensor("m_hbm1", [H1, EW], dt.bfloat16, kind="Internal")]
    # one Shared tensor holding both gathered half-tables; the gathers read
    # slices of it so they serialize behind BOTH AllGathers (overlapping the
    # gathers with the collectives measured as a net loss: the mixed
    # scattered-read + CC traffic runs below the sum of the solo rates)
    m_full_all = nc.dram_tensor("m_full_all", [NC * NSP, EW], dt.bfloat16,
                                kind="Internal", addr_space="Shared")
    m_full = [m_full_all.ap()[0:NC * H0, :],
              m_full_all.ap()[NC * H0:NC * NSP, :]]
    st_in = [nc.dram_tensor(f"st_in{i}", [6, 2], dt.float32, kind="Internal")
             for i in range(3)]
    st_out = [nc.dram_tensor(f"st_out{i}", [6, 2], dt.float32, kind="Internal",
                             addr_space="Shared") for i in range(3)]
    bar_in = nc.dram_tensor("bar_in", [1, 1], dt.float32, kind="Internal")
    bar_out = nc.dram_tensor("bar_out", [1, 1], dt.float32, kind="Internal",
                             addr_space="Shared")
    mlp_in = nc.dram_tensor("mlp_in", [2, 128, 32], dt.float32, kind="Internal")
    mlp_out = nc.dram_tensor("mlp_out", [2, 128, 32], dt.float32,
                             kind="Internal", addr_space="Shared")

    groups = [list(range(NC))]
    NSTG = (H * NBLK + 15) // 16       # lw1 staging tiles
    STG_BUFS = 5

    with tile.TileContext(nc) as tc:
        with (
            tc.tile_pool(name="const", bufs=1) as cpool,
            tc.tile_pool(name="ho", bufs=1) as ho_pool,
            tc.tile_pool(name="mfm", bufs=1) as mfm_pool,
            tc.tile_pool(name="mnm", bufs=1) as mnm_pool,
            tc.tile_pool(name="msg", bufs=10) as msg_pool,
            tc.tile_pool(name="ohp", bufs=6) as oh_pool,
            tc.tile_pool(name="ysc", bufs=2) as y_pool,
            tc.tile_pool(name="st", bufs=1) as st_pool,
            tc.tile_pool(name="t6", bufs=1) as t6_pool,
            tc.tile_pool(name="stg", bufs=STG_BUFS) as stg_pool,
            tc.tile_pool(name="psA", bufs=4, space="PSUM") as psA,
            tc.tile_pool(name="ps1", bufs=2, space="PSUM") as ps1,
            tc.tile_pool(name="psD", bufs=1, space="PSUM") as psD,
        ):
            # sync cores first so the bootstrap barrier + skew absorb into
            # the setup loads instead of delaying the first AllGather
            nc.gpsimd.collective_compute(
                "AllReduce", ALU.add, replica_groups=groups,
                ins=[bar_in.ap()], outs=[bar_out.ap()])
            # ---------- setup ----------
            gmsg_sb = []
            for tt in (0, 1):
                t = cpool.tile([128, TOK[tt] // 16], dt.int16,
                               tag=f"gmsg{tt}", name=f"gmsg_sb{tt}")
                nc.sync.dma_start(t[:], gmsg[tt][:])
                gmsg_sb.append(t)
            bw_sb = []
            for i in range(3):
                t = cpool.tile([48 if i == 0 else 96, 96], dt.bfloat16,
                               tag=f"bw{i}", name=f"bw_sb{i}")
                nc.sync.dma_start(t[:], bw[i][:])
                bw_sb.append(t)
            i96_sb = cpool.tile([96, 96], dt.bfloat16)
            nc.sync.dma_start(i96_sb[:], i96[:])
            sel_sb = cpool.tile([96, 6], dt.float32)
            nc.sync.dma_start(sel_sb[:], sel[:])
            selT_sb = cpool.tile([6, 96], dt.float32)
            nc.sync.dma_start(selT_sb[:], selT[:])
            gam_sb, bet_sb = [], []
            for i in range(3):
                g_t = cpool.tile([6, 1], dt.float32, tag=f"gam{i}",
                                 name=f"gam_sb{i}")
                nc.sync.dma_start(g_t[:], gam[i][:])
                gam_sb.append(g_t)
                b_t = cpool.tile([6, 1], dt.float32, tag=f"bet{i}",
                                 name=f"bet_sb{i}")
                nc.sync.dma_start(b_t[:], bet[i][:])
                bet_sb.append(b_t)
            eps_sb = cpool.tile([6, 1], dt.float32, name="eps_sb")
            nc.vector.memset(eps_sb[:], EPS)
            dis_sb = cpool.tile([96, NSP], dt.bfloat16)
            nc.sync.dma_start(dis_sb[:], disb[:])

            # x feature-major, already dis-scaled on host
            h_t = [ho_pool.tile([48, NSP], dt.bfloat16, tag=f"ho{u}",
                                name=f"x_sb{u}") for u in range(2)]
            nc.sync.dma_start(h_t[0][:], x0[:])
            nc.sync.dma_start(h_t[1][:], x1[:])

            def stg_load(si):
                nch = min(16, H * NBLK - si * 16)
                t = stg_pool.tile([128, nch, 256], dt.bfloat16, tag="stg",
                                  name=f"stg{si}")
                nc.scalar.dma_start(t[:], lw1q[:, si * 16:si * 16 + nch, :])
                return t

            # ================= conv layers =================
            for L in range(3):
                kin = 48 if L == 0 else 96
                m_nm = mnm_pool.tile([128, NBLK, EW], dt.bfloat16, tag="mnm",
                                     name=f"mnm_L{L}")
                if L == 0:
                    nc.vector.memset(m_nm[:, :, W:EW], 0.0)

                def emit_mnm(c0, c1, on_group, L=L, m_nm=m_nm, h_t=h_t):
                    for b0 in range(c0, c1, 5):
                        nb = min(5, c1 - b0)
                        for u in range(2):
                            pt = ps1.tile([128, 512], dt.float32, tag="ps1",
                                          name=f"ptb_L{L}")
                            for i in range(nb):
                                c = b0 + i
                                nc.tensor.matmul(
                                    pt[:, 96 * i:96 * (i + 1)],
                                    h_t[u][:, 128 * c:128 * (c + 1)],
                                    bw_sb[L][:], start=True, stop=True)
                            src = (pt[:, :96 * nb]
                                   .rearrange("p (c f) -> p c f", f=96))
                            nc.vector.tensor_copy(
                                m_nm[:, b0:b0 + nb, 96 * u:96 * (u + 1)], src)
                        on_group(b0, nb)

                # half A -> AllGather0, half B -> AllGather1; the m_hbm
                # writes go out per 5-chunk group, overlapping the transform
                def m_write(b0, nb, L=L, m_nm=m_nm):
                    half = 0 if b0 < H0C else 1
                    base = b0 - (0 if half == 0 else H0C)
                    nc.sync.dma_start(
                        m_hbm[half].ap()[128 * base:128 * (base + nb), :]
                        .rearrange("(c p) e -> p c e", p=128),
                        m_nm[:, b0:b0 + nb, :])

                emit_mnm(0, H0C, on_group=m_write)
                nc.gpsimd.collective_compute(
                    "AllGather", ALU.bypass, replica_groups=groups,
                    ins=[m_hbm[0].ap()], outs=[m_full[0]])
                emit_mnm(H0C, NBLK, on_group=m_write)
                nc.gpsimd.collective_compute(
                    "AllGather", ALU.bypass, replica_groups=groups,
                    ins=[m_hbm[1].ap()], outs=[m_full[1]])

                # m_fm (feature-major self term) runs under the AllGathers
                m_fm = [mfm_pool.tile([96, NSP], dt.bfloat16, tag=f"mfm{u}",
                                      name=f"mfm_L{L}_{u}") for u in range(2)]
                for u in range(2):
                    for c0 in range(0, NSP, 512):
                        cw = min(512, NSP - c0)
                        pt = ps1.tile([128, 512], dt.float32, tag="ps1",
                                      name=f"ptf_L{L}")
                        nc.tensor.matmul(pt[0:96, :cw], bw_sb[L][:],
                                         h_t[u][:, c0:c0 + cw],
                                         start=True, stop=True)
                        nc.vector.tensor_copy(m_fm[u][:, c0:c0 + cw],
                                              pt[0:96, :cw])

                # prefetch lw1 staging during L3's gather passes
                stg_tiles = {}
                if L == 2:
                    for si in range(STG_BUFS):
                        stg_tiles[si] = stg_load(si)

                # gather calls + one-hot loads, emitted in consumption order
                # (sorted by the first window each one feeds)
                def first_window(tt, q0):
                    jb = int(np.searchsorted(CHOFF[tt], q0, side="right")) - 1
                    return jb // BLK_PER_W

                calls = sorted(
                    [(first_window(tt, ci * CPC), tt, ci)
                     for tt in (0, 1) for ci in range(NCALL[tt])])
                msg_tiles = [[None] * NCALL[tt] for tt in (0, 1)]
                for qi, (_, tt, ci) in enumerate(calls):
                    nch = min(CPC, NCHUNK[tt] - ci * CPC)
                    t = msg_pool.tile([128, nch, EW], dt.bfloat16,
                                      tag="msg", name=f"msg_L{L}_{tt}_{ci}")
                    nc.gpsimd.dma_gather(
                        t[:], m_full[tt],
                        gmsg_sb[tt][:, ci * (CALL // 16):
                                    ci * (CALL // 16) + nch * 8],
                        num_idxs=nch * 128, num_idxs_reg=nch * 128,
                        elem_size=EW, queue_num=qi % NQ)
                    msg_tiles[tt][ci] = t
                ohs = sorted(
                    [(first_window(tt, ti * 16), tt, ti)
                     for tt in (0, 1) for ti in range(NOHT[tt])])
                oh_tiles = [[None] * NOHT[tt] for tt in (0, 1)]
                for _, tt, ti in ohs:
                    nch = min(16, NCHUNK[tt] - ti * 16)
                    t = oh_pool.tile([128, nch, 128], dt.float8e4,
                                     tag="oh", name=f"oh_L{L}_{tt}_{ti}")
                    nc.sync.dma_start(
                        t[:], oh_in[tt][:, ti * 16:ti * 16 + nch, :])
                    oh_tiles[tt][ti] = t

                o_t = [ho_pool.tile([96, NSP], dt.bfloat16, tag=f"ho{u}",
                                    name=f"o_L{L}_{u}") for u in range(2)]
                S_t = st_pool.tile([96, 4 * NW], dt.float32, tag="S")

                def seg_window(w, L=L, msg_tiles=msg_tiles,
                               oh_tiles=oh_tiles, o_t=o_t,
                               m_fm=m_fm, S_t=S_t):
                    jlo = w * BLK_PER_W
                    jhi = min(jlo + BLK_PER_W, NBLK)
                    pw = [psA.tile([96, 512], dt.float32, tag="psA",
                                   name=f"pw_L{L}_{w}_{uu}")
                          for uu in range(2)]
                    for j in range(jlo, jhi):
                        qs = [(tt, int(CHOFF[tt][j]) + c)
                              for tt in (0, 1) for c in range(int(CH[tt][j]))]
                        for k, (tt, q) in enumerate(qs):
                            mt = msg_tiles[tt][q // CPC]
                            ot = oh_tiles[tt][q // 16]
                            for u in range(2):
                                nc.tensor.matmul(
                                    pw[u][:, 128 * (j - jlo):
                                          128 * (j - jlo + 1)],
                                    mt[:, q % CPC, 96 * u:96 * (u + 1)],
                                    ot[:, q % 16, :],
                                    start=(k == 0),
                                    stop=(k == len(qs) - 1))
                    c0 = 512 * w
                    cw = min(512, NS - c0)    # stats over real nodes only
                    cwf = min(512, NSP - c0)
                    for u in range(2):
                        y = y_pool.tile([96, 512], dt.float32, tag="y")
                        nc.vector.tensor_add(
                            y[:, :cwf], pw[u][:, :cwf], m_fm[u][:, c0:c0 + cwf])
                        nc.vector.tensor_mul(
                            o_t[u][:, c0:c0 + cwf], y[:, :cwf],
                            dis_sb[:, c0:c0 + cwf])
                        # stats: sum + sum-of-squares via Act accumulators
                        y2 = y_pool.tile([96, 512], dt.bfloat16, tag="y2")
                        nc.scalar.activation(
                            y2[:, :cw], o_t[u][:, c0:c0 + cw], AF.Copy,
                            accum_out=S_t[:, 2 * w + u:2 * w + u + 1])
                        nc.scalar.activation(
                            y2[:, :cw], o_t[u][:, c0:c0 + cw], AF.Square,
                            accum_out=S_t[:, 2 * (NW + w) + u:
                                          2 * (NW + w) + u + 1])

                for w in range(NW):
                    seg_window(w)

                # ---------- BN stats + apply ----------
                st2 = st_pool.tile([96, 4], dt.float32, tag="st2")
                for u in range(2):
                    nc.vector.tensor_reduce(
                        st2[:, u:u + 1],
                        S_t[:, :2 * NW].rearrange("p (w u) -> p u w", u=2)
                        [:, u, :], axis=mybir.AxisListType.X, op=ALU.add)
                    nc.vector.tensor_reduce(
                        st2[:, 2 + u:3 + u],
                        S_t[:, 2 * NW:4 * NW]
                        .rearrange("p (w u) -> p u w", u=2)[:, u, :],
                        axis=mybir.AxisListType.X, op=ALU.add)
                pst = psD.tile([6, 2], dt.float32, tag="pst")
                for u in range(2):
                    nc.tensor.matmul(
                        pst[:], sel_sb[:],
                        st2[:, :].rearrange("p (a u) -> p u a", u=2)[:, u, :],
                        start=(u == 0), stop=(u == 1))
                stt = t6_pool.tile([6, 2], dt.float32, tag="stt")
                nc.vector.tensor_copy(stt[:], pst[:])
                nc.sync.dma_start(st_in[L].ap(), stt[:])
                nc.gpsimd.collective_compute(
                    "AllReduce", ALU.add, replica_groups=groups,
                    ins=[st_in[L].ap()], outs=[st_out[L].ap()])
                sto = t6_pool.tile([6, 2], dt.float32, tag="sto")
                nc.sync.dma_start(sto[:], st_out[L].ap())
                # sel carries 1/NB, so sto = [mu, E[y^2]]
                musq = t6_pool.tile([6, 1], dt.float32, tag="musq")
                nc.vector.tensor_mul(musq[:], sto[:, 0:1], sto[:, 0:1])
                var = t6_pool.tile([6, 1], dt.float32, tag="var")
                nc.vector.tensor_sub(var[:], sto[:, 1:2], musq[:])
                nc.scalar.activation(var[:], var[:], AF.Sqrt,
                                     bias=eps_sb[:].opt())
                nc.vector.reciprocal(var[:], var[:])     # var := 1/sigma
                ab6 = t6_pool.tile([6, 2], dt.float32, tag="ab6")
                nc.vector.tensor_mul(ab6[:, 0:1], gam_sb[L][:], var[:])
                nc.vector.tensor_mul(musq[:], sto[:, 0:1], ab6[:, 0:1])
                nc.vector.tensor_sub(ab6[:, 1:2], bet_sb[L][:], musq[:])
                pab = psD.tile([96, 2], dt.float32, tag="pab")
                nc.tensor.matmul(pab[:], selT_sb[:], ab6[:],
                                 start=True, stop=True)
                ab = st_pool.tile([96, 2], dt.float32, tag="ab")
                nc.vector.tensor_copy(ab[:], pab[:])
                # BN apply + relu (in place); fold dis for the next conv
                for u in range(2):
                    nc.scalar.activation(o_t[u][:], o_t[u][:], AF.Relu,
                                         bias=ab[:, 1:2].opt(),
                                         scale=ab[:, 0:1].opt())
                    if L < 2:
                        nc.vector.tensor_mul(o_t[u][:], o_t[u][:], dis_sb[:])
                h_t = [o_t[0], o_t[1]]

            # ================= MLP head =================
            # o -> node-major bf16 (transpose via PE with identity)
            o_bf = mnm_pool.tile([128, NBLK, EW], dt.bfloat16, tag="mnm",
                                 name="o_bf")
            for u in range(2):
                for b0 in range(0, NBLK, 5):
                    nb = min(5, NBLK - b0)
                    pt = ps1.tile([128, 512], dt.float32, tag="ps1",
                                  name="pto")
                    for i in range(nb):
                        c = b0 + i
                        nc.tensor.matmul(pt[:, 96 * i:96 * (i + 1)],
                                         h_t[u][:, 128 * c:128 * (c + 1)],
                                         i96_sb[:], start=True, stop=True)
                    src = pt[:, :96 * nb].rearrange("p (c f) -> p c f", f=96)
                    nc.vector.tensor_copy(
                        o_bf[:, b0:b0 + nb, 96 * u:96 * (u + 1)], src)

            zt = [psD.tile([128, 32], dt.float32, tag="pst", name="zt0"),
                  psD.tile([128, 32], dt.float32, tag="pab", name="zt1")]
            for si in range(STG_BUFS, NSTG):
                stg_tiles[si] = stg_load(si)
            for f in range(H):
                for c in range(NBLK):
                    fc = f * NBLK + c
                    st_t = stg_tiles[fc // 16]
                    rhs = (o_bf[:, c, 0:W]
                           .rearrange("p (g f) -> p f g", f=H)[:, f, :])
                    for h2 in range(2):
                        nc.tensor.matmul(
                            zt[h2][:],
                            st_t[:, fc % 16, 128 * h2:128 * (h2 + 1)],
                            rhs, start=(fc == 0), stop=(fc == H * NBLK - 1))
            zc = [st_pool.tile([128, 32], dt.float32, tag=f"zc{h2}",
                               name=f"zc{h2}") for h2 in range(2)]
            for h2 in range(2):
                nc.vector.tensor_copy(zc[h2][:], zt[h2][:])
                nc.sync.dma_start(mlp_in.ap()[h2], zc[h2][:])
            nc.gpsimd.collective_compute(
                "AllReduce", ALU.add, replica_groups=groups,
                ins=[mlp_in.ap()], outs=[mlp_out.ap()])
            lb1_sb = cpool.tile([128, 2], dt.float32)
            nc.sync.dma_start(lb1_sb[:], lb1h[:])
            lw2_sb = cpool.tile([128, 2, 128], dt.float32)
            nc.sync.dma_start(lw2_sb[:], lw2r[:])
            lw3_sb = cpool.tile([128, 64], dt.float32)
            nc.sync.dma_start(lw3_sb[:], lw3[:])
            lw4_sb = cpool.tile([64, 32], dt.float32)
            nc.sync.dma_start(lw4_sb[:], lw4[:])
            lb2_sb = cpool.tile([128, 1], dt.float32)
            nc.sync.dma_start(lb2_sb[:], lb2c[:])
            lb3_sb = cpool.tile([64, 1], dt.float32)
            nc.sync.dma_start(lb3_sb[:], lb3c[:])
            lb4_sb = cpool.tile([32, 1], dt.float32)
            nc.sync.dma_start(lb4_sb[:], lb4c[:])

            h1 = [st_pool.tile([128, 32], dt.float32, tag=f"h1{h2}",
                               name=f"h1_{h2}") for h2 in range(2)]
            for h2 in range(2):
                nc.sync.dma_start(h1[h2][:], mlp_out.ap()[h2])
                nc.scalar.activation(h1[h2][:], h1[h2][:], AF.Relu,
                                     bias=lb1_sb[:, h2:h2 + 1].opt())
            p2 = psD.tile([128, 32], dt.float32, tag="pst")
            for h2 in range(2):
                nc.tensor.matmul(p2[:], lw2_sb[:, h2, :], h1[h2][:],
                                 start=(h2 == 0), stop=(h2 == 1))
            h2x = st_pool.tile([128, 32], dt.float32, tag="h2x")
            nc.scalar.activation(h2x[:], p2[:], AF.Relu, bias=lb2_sb[:].opt())
            p3 = psD.tile([64, 32], dt.float32, tag="pab")
            nc.tensor.matmul(p3[:], lw3_sb[:], h2x[:], start=True, stop=True)
            h3 = st_pool.tile([64, 32], dt.float32, tag="h3")
            nc.scalar.activation(h3[:], p3[:], AF.Relu, bias=lb3_sb[:].opt())
            p4 = psD.tile([32, 32], dt.float32, tag="pst")
            nc.tensor.matmul(p4[:], lw4_sb[:], h3[:], start=True, stop=True)
            o4 = st_pool.tile([32, 32], dt.float32, tag="o4")
            nc.vector.tensor_scalar_add(o4[:], p4[:], lb4_sb[:].opt())
            nc.sync.dma_start(out_d.ap().rearrange("g c -> c g"), o4[:])

    nc.compile()
    return nc


def _make_in_maps(plan, inputs):
    f32 = np.float32
    bf16 = ml_dtypes.bfloat16
    x = np.asarray(inputs["x"], f32)
    W1 = np.asarray(inputs["W1"], f32)
    W2 = np.asarray(inputs["W2"], f32)
    W3 = np.asarray(inputs["W3"], f32)
    lw1 = np.asarray(inputs["lw1"], f32)
    lw2 = np.asarray(inputs["lw2"], f32)
    lw3 = np.asarray(inputs["lw3"], f32)
    lw4 = np.asarray(inputs["lw4"], f32)

    bw1 = np.kron(np.eye(16, dtype=f32), W1).astype(bf16)   # [48, 96]
    bw2 = np.kron(np.eye(16, dtype=f32), W2).astype(bf16)   # [96, 96]
    bw3 = np.kron(np.eye(16, dtype=f32), W3).astype(bf16)
    i96 = np.eye(96, dtype=f32).astype(bf16)
    sel = np.tile(np.eye(6, dtype=f32) / (B * N), (16, 1))   # [96, 6] (/NB)
    selT = np.ascontiguousarray(np.tile(np.eye(6, dtype=f32), (16, 1)).T)
    lw2r = np.ascontiguousarray(
        lw2.reshape(2, 128, 128).transpose(1, 0, 2))         # [128, 2, 128]
    lb1h = np.ascontiguousarray(
        np.asarray(inputs["lb1"], f32).reshape(2, 128).T)    # [128, 2]

    dis = plan["dis"]                                        # [N]
    xg = x.reshape(B, N, DIM)

    common = {
        "bw1": bw1, "bw2": bw2, "bw3": bw3, "i96": i96,
        "sel": sel, "selT": selT,
        "g1": np.asarray(inputs["g1"], f32).reshape(6, 1),
        "be1": np.asarray(inputs["be1"], f32).reshape(6, 1),
        "g2": np.asarray(inputs["g2"], f32).reshape(6, 1),
        "be2": np.asarray(inputs["be2"], f32).reshape(6, 1),
        "g3": np.asarray(inputs["g3"], f32).reshape(6, 1),
        "be3": np.asarray(inputs["be3"], f32).reshape(6, 1),
        "lw2r": lw2r, "lw3": lw3, "lw4": lw4,
        "lb1h": lb1h,
        "lb2c": np.asarray(inputs["lb2"], f32).reshape(128, 1),
        "lb3c": np.asarray(inputs["lb3"], f32).reshape(64, 1),
        "lb4c": np.asarray(inputs["lb4"], f32).reshape(32, 1),
    }

    in_maps = []
    for k in range(NC):
        pc = plan["per_core"][k]
        dk = dis[NS * k:NS * (k + 1)]
        # x feature-major, dis-folded: xq[u][g*3+fi, n]
        xs = xg[:, NS * k:NS * (k + 1), :]              # [32, 6250, 3]
        xq = np.zeros((2, 48, NSP), f32)
        for u in range(2):
            blkv = (xs[16 * u:16 * (u + 1)] * dk[None, :, None])
            xq[u, :, :NS] = blkv.transpose(0, 2, 1).reshape(48, NS)
        disb = np.ones((96, NSP), f32)
        disb[:, :NS] = dk[None, :]
        # lw1 as [partition, (f, chunk), 256] so each staging tile is one
        # contiguous 8KB run per partition
        lw1s = lw1[NS * k * H:NS * (k + 1) * H].reshape(NS, H, 256)
        lw1f = np.zeros((H, NSP, 256), np.float32)
        lw1f[:, :NS, :] = lw1s.transpose(1, 0, 2)
        # [H, NBLK, 128, 256] -> [128, H*NBLK, 256]
        lw1q = (lw1f.reshape(H, NBLK, 128, 256).transpose(2, 0, 1, 3)
                .reshape(128, H * NBLK, 256).astype(bf16))
        m = dict(common)
        m.update({
            "x0": np.ascontiguousarray(xq[0]).astype(bf16),
            "x1": np.ascontiguousarray(xq[1]).astype(bf16),
            "disb": disb.astype(bf16),
            "gmsg0": pc["gmsg0"], "gmsg1": pc["gmsg1"],
            "oh0": pc["oh0"], "oh1": pc["oh1"],
            "lw1q": np.ascontiguousarray(lw1q),
        })
        in_maps.append(m)
    return in_maps


def _get(edge_base):
    key = hash(np.asarray(edge_base).tobytes())
    if key not in _cache:
        plan = _build_plan(np.asarray(edge_base))
        nc = _build_nc(plan)
        _cache[key] = (plan, nc)
    return _cache[key]


def kernel(**inputs):
    from concourse.bass_utils import run_bass_kernel_spmd
    assert int(inputs["num_graphs"]) == B and int(inputs["num_nodes"]) == N
    plan, nc = _get(inputs["edge_base"])
    in_maps = _make_in_maps(plan, inputs)
    trace = os.environ.get("KERNEL_TRACE", "0") == "1"
    res = run_bass_kernel_spmd(nc, in_maps, core_ids=list(range(NC)),
                               trace=trace)
    kernel.last_result = res
    return np.ascontiguousarray(res.results[0]["out"])
